# revision 44
# baseline (speedup 1.0000x reference)
"""Trainium2 Bass kernel for nn_BMMS8TS8NS8T: batched int8-valued GEMM with
dequant/requant, sharded head-parallel across 8 NeuronCores.

Reference semantics (jax CPU, fp32):
    a = x.float() - a_zp          # [B,H,S,D]  int8-valued
    b = y.float() - b_zp          # [B,H,D,T]
    q = a @ b                     # exact integers (|q| <= 64*131*132 < 2^24)
    v = fl(fl(q * s) + out_zp),   s = fl(alpha / o_alpha)
    out = trunc(clip(v, -128, 127)).astype(int8)   # trunc toward zero

Device strategy per core (12 heads = (B*H)/8, head parallel, no cross-core
communication):
  - host pre-dequantizes inputs to bf16 (exact: integers with |v| <= 132 are
    exactly representable) and pre-transposes/permutes x so the stationary
    matmul operand needs no on-device transpose; x^T and y are packed into
    ONE dram tensor per head-pair so each pair costs one input dma_start
  - TensorE: K=64 matmuls, two heads packed in the 128-row PE array via row
    tiling (tile_position (0,0)/(64,0)); fp32 PSUM accumulation is exact.
    The two heads' matmuls are emitted INTERLEAVED (H2_ILV) so adjacent PE
    instructions land on different row-groups and execute concurrently
    (~2x effective PE rate, earlier drain starts; measured -4us/iter)
  - PSUM as 4 rotating [128,1024] quads (bufs=4) so matmul fills decouple
    from drains (2x[128,2048] ping-pong was period-bound: a tile's refill
    waited on its own ~2.1us drain -> ~71us/iter cap)
  - requantization: the key insight is a CUSTOM DVE uop (per-NEFF uop table,
    registered at import into concourse.dve_ops) that does the whole
    requant+trunc in ONE 1x pass from PSUM fp32 straight to int8:
        u   = fl(fl(q * s2) + zp),  s2 = fl(s * (1 + 2^-22))
        out = sat_i8(RNE(u + 0.5*((u < -0.5) - (u > 0.5))))
    The three-zone +-0.5 shift makes RNE produce trunc-toward-zero incl.
    the double-width bin at 0 that no single affine+RNE can express; the
    2^-22 scale inflation breaks RNE half-to-even ties symmetrically
    (correct for trunc on both signs).  Validated exhaustively on host over
    every reachable q: 0 mismatches (saturation included); hardware runs
    bit-exact vs the jax reference.
  - 1x PSUM-read capacity is the structural floor (GpSimd has NO PSUM port;
    fp32 PSUM reads are 1x on ScalarE/VectorE -> >= 12.6M reads /
    276G elem/s ~= 46us/iter minimum).  To use all three post-PE engines,
    ACT_G_OF_48 groups take a two-pass path (ScalarE pass1 -> int16,
    GpSimd/Pool pass2 tensor_scalar A*(255/256)+0.499) and the rest use the
    fused VectorE op.  Measured balance ~51/57/45us (ACT/DVE/Pool) per
    iteration; Pool ops stretch under DVE shared-SBUF-port contention, so
    pushing pass2 work to DVE (STT or a 2x custom op) measured WORSE.
  - x^T columns are host-permuted so psum partition p owns output rows
    s = 8p+j: a whole head's output is one [128, 8192] int8 staging tile
    stored with a single dma_start (dma_start costs ~0.6-4us of sequencer
    descriptor-generation time; 60 small DMAs saturated the SP ring, and
    alternating rings (DMA_ALT) stalls ScalarE compute - keep all on SP)
Loop-slope A/B (hardware, same-process interleaved, min-of-reps):
    v1 two-pass split-engine baseline:    96-102 us/iter (graded 548503 ns)
    + 4-quad psum, Pool pass2, fused op:  60-76 us/iter
    + h2-interleaved matmul emission:     ~1.06x further (final 64.5 us/iter
      with exact output in the same test.py run; absolute scale drifts
      ~25% between runs - within-run ratios are the trusted signal)
"""

from contextlib import ExitStack
import numpy as np
import ml_dtypes

import concourse.bacc as bacc
import concourse.tile as tile
from concourse import mybir
from concourse import dve_ops as _dve_ops
from concourse.bass_utils import run_bass_kernel_spmd
from concourse.dve_spec import C0, C1, C2, Spec, Src0, Zero, lower
from concourse.dve_uop import DveOpSpec

AF = mybir.ActivationFunctionType
OP = mybir.AluOpType
BF16 = mybir.dt.bfloat16


def _trunc_requant_ref(in0, in1, s0, s1, imm2):
    """Faithful fp32 emulation of TRUNC_REQUANT_ANT for CoreSim."""
    f32 = np.float32
    u = (in0.astype(f32) * f32(s0)).astype(f32) + f32(s1)
    u = u.astype(f32)
    ind = (u < f32(-imm2)).astype(f32) - (u > f32(imm2)).astype(f32)
    return (u + (f32(imm2) * ind).astype(f32)).astype(f32)


def _register_trunc_op():
    """Custom DVE uop: single-pass exact requant+trunc, psum fp32 -> int8.

        u   = fl(fl(q * s2) + zp)               (s2 = s * (1 + 2^-22))
        out = sat_i8(RNE(u + 0.5*((u < -0.5) - (u > 0.5))))

    The three-zone correction shifts u by +-0.5 so RNE lands on
    trunc-toward-zero; the middle zone gives the double-width output bin at
    0 that no single affine+RNE can produce.  Validated exhaustively on the
    host over every reachable q in [-1106688, 1047552]: 0 mismatches vs the
    reference fp32 chain (including saturation; RNE half-to-even ties fixed
    by the 2^-22 scale inflation, symmetric for trunc).
    """
    name = "TRUNC_REQUANT_ANT"
    for op in _dve_ops.OPS:
        if op.name == name:
            return op
    _u = Src0 * C0 + C1
    spec = Spec(
        body=_u + C2 * ((_u < -C2) - (_u > C2)),
        reference=_trunc_requant_ref,
    )
    row = max(_dve_ops._SUB_OPCODE_FOR_NAME.values()) + 1
    assert row < 0x20
    shas = {}
    for ver in ("v3", "v4"):
        try:
            uops = lower(spec, ver=ver)
            shas[ver] = DveOpSpec(name=name, opcode=row, uops=uops,
                                  rd1_en=False).sha(ver)
        except Exception:
            pass
    op = _dve_ops.DveOp(name=name, spec=spec, subdim=False, uops_sha=shas)
    _dve_ops.OPS.append(op)
    _dve_ops.CUSTOM_DVE_SPECS[name] = spec
    _dve_ops._SUB_OPCODE_FOR_NAME[name] = row
    return op


TRUNC_OP = _register_trunc_op()


def _pass2_ref(in0, in1, s0, s1, imm2):
    f32 = np.float32
    a = in0.astype(f32)
    return (a + (a < f32(0.0)).astype(f32)).astype(f32)


def _register_pass2_op():
    """Custom DVE uop for pass2: out_i8 = sat(RNE(A + [A<0])), A int16.
    Single-src, 16-bit -> eligible for the 2x_1P perf slot (perf_en), which
    doesn't touch the DVE/Pool shared SBUF port pair."""
    name = "P2_TRUNC_ANT"
    for op in _dve_ops.OPS:
        if op.name == name:
            return op
    spec = Spec(body=Src0 + (Src0 < Zero), reference=_pass2_ref)
    row = max(_dve_ops._SUB_OPCODE_FOR_NAME.values()) + 1
    assert row < 0x20
    shas = {}
    perf = {}
    for ver in ("v3", "v4"):
        try:
            uops = lower(spec, ver=ver)
            shas[ver] = DveOpSpec(name=name, opcode=row, uops=uops,
                                  rd1_en=False).sha(ver)
            perf[ver] = True
        except Exception:
            pass
    op = _dve_ops.DveOp(name=name, spec=spec, subdim=False, uops_sha=shas,
                        perf_en=perf)
    _dve_ops.OPS.append(op)
    _dve_ops.CUSTOM_DVE_SPECS[name] = spec
    _dve_ops._SUB_OPCODE_FOR_NAME[name] = row
    return op


P2_OP = _register_pass2_op()

N_CORES = 8
B, H, S, D = 8, 12, 1024, 64
HEADS_PER_CORE = B * H // N_CORES          # 12
N_PAIRS = HEADS_PER_CORE // 2              # 6
M_BLOCKS = S // 128                        # 8
T = 1024

# set by kernel() for test.py / bench.py to inspect
LAST_RESULTS = None
LAST_PREP = None

_NC_CACHE = {}

# --- engine-assignment knobs (all values hardware-A/B'd; see docstring)
H2_ILV = True           # interleave h0/h1 matmul emission for PE row-group
                        # concurrency (PE 41us -> ~21us busy; also removes
                        # any HAM rethrottle risk from PE idle gaps)
P2_DVE_OF = 0           # ACT-group pass2 ops moved to the DVE 2x custom op
                        # (measured monotonically WORSE: 4->+21us, 25->+46us
                        # per iter - DVE is the co-busiest engine)
SPLIT_JI = False        # per-group quad split ji0->ACT+Pool / ji1->fused
                        # DVE (measured +19us/iter worse: every group having
                        # adjacent DVE+Pool ops maximizes the shared-port
                        # contention stretch on Pool)
ACT_G_OF_48 = 25        # of the 48 groups, how many take the ScalarE
                        # two-pass path (ACT pass1 int16 + Pool pass2);
                        # the rest use the fused single-pass custom DVE op
                        # (TRUNC_REQUANT_ANT, psum fp32 -> int8 directly)
A_BUFS = 6              # [128, 2048] int16 pass1 staging tiles
ACT_BY_H2 = False       # group->engine by head instead of Bresenham:
                        # h0 -> ACT two-pass, h1 -> fused DVE (24/24) so the
                        # two concurrent drains of every j-block are always
                        # on different engines
P2_ACT_OF = 0           # of the ACT-groups' pass2 ops, how many run on
                        # ScalarE (activation Copy(A*c_b + d_b), identical
                        # numerics) instead of Pool - trades stretched Pool
                        # time (~2.9us/op under DVE port contention) for ACT
                        # slack (~2.0us/op)
P2_HALVES = False       # Pool pass2 as two [128,1024] ops per group (can
                        # start after the first ACT quad-drain; finer Pool
                        # queue granularity) vs one [128,2048] op
OB_BUFS = 4             # [128, 8192] int8 staging tiles (one per head;
                        # 4th buffer relieves a store-WAR stall under the
                        # interleaved emission - measured -4.7us/iter)
XIN_BUFS = 2            # packed [128, 2048] bf16 input tiles
DMA_ALT = False         # alternate dma_start between the two HWDGE rings
                        # (SP qSPDynamicHW / ACT qActDynamicHW) - descriptor
                        # generation is ~1.7-4us of sequencer time per DMA
                        # and saturates a single ring
DEFER_N = 2             # how many groups the pass2 trails its pass1
NH_SPLIT = 2            # rhs blocks per ji (N=512 moving operand)
SPLIT_FIRST_LOAD = True

# bench-only ablation knobs (correctness-invalid when set; used by ab.py to
# attribute loop-slope time to pipeline stages)
BENCH_NO_STORE = False
BENCH_NO_LOAD = False
BENCH_NO_MM = False
BENCH_NO_P2 = False
BENCH_NO_P1 = False


def _build_core_program(s_const: float, bias_a: float, c_b: float, d_b: float,
                        loop_iters: int | None = None,
                        bench_io: bool = False):
    """One NeuronCore's program: 12 heads of [1024,64]@[64,1024] + requant.

    loop_iters: when set, wraps the whole body in a hardware For_i loop -
    used only for benchmarking (device time scales with the loop count so a
    slope isolates HW exec time from host/relay dispatch overhead).

    bench_io: all big DRAM tensors become Internal (device-resident garbage,
    never shipped over the axon relay) and a tiny [1,16] ExternalOutput is
    added so PJRT has something to return.  Timing is data-independent, so
    the loop slope is unaffected; per-call payload drops from ~15 MB/core to
    16 bytes.
    """
    nc = bacc.Bacc("TRN2", target_bir_lowering=False, debug=False)
    io_kind_in = "Internal" if bench_io else "ExternalInput"
    io_kind_out = "Internal" if bench_io else "ExternalOutput"
    # head-pairs stacked on the partition axis; xt and yp packed into one
    # tensor so each pair needs a single input dma_start (SP sequencer DGE
    # config is ~1.7us per dma_start - the dominant serial cost at 60 DMAs)
    d_xy = nc.dram_tensor("xy", [N_PAIRS, 128, S + T], BF16, kind=io_kind_in)
    d_o = nc.dram_tensor("o", [HEADS_PER_CORE, S, T], mybir.dt.int8,
                         kind=io_kind_out)
    d_tick = (nc.dram_tensor("tick", [1, 16], mybir.dt.int8,
                             kind="ExternalOutput") if bench_io else None)

    with tile.TileContext(nc) as tc:
        with ExitStack() as stk:
            if loop_iters is not None:
                # PE's body exceeds one IRAM block; hint the back-edge so the
                # benchmark loop doesn't pay a ~3-4 us ifetch per iteration
                # that single-shot execution would not pay.
                stk.enter_context(tc.For_i(0, loop_iters, 1,
                                           hint_engines=(mybir.EngineType.PE,)))
            _emit_body(nc, tc, d_xy, d_o, s_const, bias_a, c_b, d_b)
        if d_tick is not None:
            with tc.tile_pool(name="tick", bufs=1) as tkpool:
                tk = tkpool.tile([1, 16], mybir.dt.int8, tag="tick")
                nc.vector.memset(tk[:], 0)
                nc.sync.dma_start(d_tick[:, :], tk[:])
    nc.compile()
    return nc


def _emit_body(nc, tc, d_xy, d_o, s_const, bias_a, c_b, d_b):
    # VectorE one-op pass1 constants (validated in validate_requant.py over
    # every reachable q: 0 mismatches, max tie margin variant)
    s64 = np.float64(np.float32(s_const))
    zp64 = np.float64(np.float32(bias_a)) + 0.5 - 2.0 ** -18  # recover out_zp
    b0 = float(np.float32((zp64 - 0.5) / s64 + 2.0 ** -8))
    s2 = float(np.float32(s64 * (1.0 + 2.0 ** -22)))

    # fused-op constants (exhaustively validated, see _register_trunc_op)
    s2c = float(np.float32(np.float64(np.float32(s_const)) *
                           (1.0 + 2.0 ** -22)))
    b2c = float(np.float32(np.float64(np.float32(bias_a)) + 0.5 - 2.0 ** -18))

    gidx = 0            # group counter (0..47)
    act_g = 0
    pending = []
    dma_i = [0]

    def dma(dst, src):
        # alternate between the two HWDGE rings; Tile still tracks deps
        eng = nc.sync if (not DMA_ALT or dma_i[0] % 2 == 0) else nc.scalar
        dma_i[0] += 1
        eng.dma_start(dst, src)
    with tc.tile_pool(name="xin", bufs=XIN_BUFS) as xpool, \
         tc.tile_pool(name="aint", bufs=A_BUFS) as apool, \
         tc.tile_pool(name="obuf", bufs=OB_BUFS) as opool, \
         tc.tile_pool(name="ps", bufs=4, space="PSUM") as pspool:
        tiles = [None] * N_PAIRS
        xy0 = xpool.tile([128, S + T], BF16, tag="xy")
        if BENCH_NO_LOAD:
            # bench-only: no HBM input traffic; one memset tile feeds every
            # head-pair (DVE ~0.9us/iter pollution, noted in readings)
            nc.vector.memset(xy0[:], 0)
        elif SPLIT_FIRST_LOAD:
            # single-shot prologue: land group-0's operands first so the
            # first matmuls start earlier (xt cols 0:256 + all of yp)
            dma(xy0[:, 0:256], d_xy[0, :, 0:256])
            dma(xy0[:, S:S + T], d_xy[0, :, S:S + T])
            dma(xy0[:, 256:S], d_xy[0, :, 256:S])
        else:
            dma(xy0[:], d_xy[0, :, :])
        tiles[0] = xy0

        ps_dummy = None
        if BENCH_NO_MM:
            # bench-only: pass1 reads this one pre-written quad (keeps the
            # psum-read cost while removing the matmul fills)
            ps_dummy = pspool.tile([128, 1024], mybir.dt.float32, tag="ps")
            nc.vector.memset(ps_dummy[:], 0)

        for pair in range(N_PAIRS):
            xy_t = tiles[pair]
            xt_t = xy_t[:, 0:S]
            yp_t = xy_t[:, S:S + T]
            if pair + 1 < N_PAIRS:
                # prefetch next pair's operands now so the loads sit ahead
                # of this pair's output stores in the SP HWDGE FIFO
                if BENCH_NO_LOAD:
                    tiles[pair + 1] = xy0
                else:
                    xy_n = xpool.tile([128, S + T], BF16, tag="xy")
                    dma(xy_n[:], d_xy[pair + 1, :, :])
                    tiles[pair + 1] = xy_n

            ob = {}

            for jg in range(M_BLOCKS // 2):
                if H2_ILV:
                    # interleave the two heads' matmuls so adjacent PE
                    # instructions target different row-groups (0,0)/(64,0)
                    # and execute concurrently (~2x effective PE rate);
                    # drains per quad follow right after its two matmuls.
                    gs = []
                    for h2 in range(2):
                        if ACT_BY_H2:
                            want_act = (h2 == 0)
                            gidx += 1
                            if want_act:
                                act_g += 1
                        else:
                            want_act = ((gidx + 1) * ACT_G_OF_48) // 48 \
                                > act_g
                            gidx += 1
                            if want_act:
                                act_g += 1
                        a_t = (apool.tile([128, 2048], mybir.dt.int16,
                                          tag="a",
                                          name=f"a_{pair}_{h2}_{jg}")
                               if want_act else None)
                        if h2 not in ob:
                            ob[h2] = opool.tile([128, 8192], mybir.dt.int8,
                                                tag="obs",
                                                name=f"ob_{pair}_{h2}")
                        ob_t = ob[h2][:, jg * 2048:(jg + 1) * 2048]
                        gs.append((h2, want_act, a_t, ob_t))
                    for ji in range(2):
                        j = jg * 2 + ji
                        psjs = {}
                        for h2, _, _, _ in gs:
                            psjs[h2] = pspool.tile(
                                [128, 1024], mybir.dt.float32, tag="ps",
                                name=f"ps_{pair}_{h2}_{jg}_{ji}")
                        for nh in range(NH_SPLIT):
                            nw = 1024 // NH_SPLIT
                            for h2, _, _, _ in gs:
                                nc.tensor.matmul(
                                    psjs[h2][:, nh * nw:(nh + 1) * nw],
                                    xt_t[64 * h2:64 * h2 + 64,
                                         j * 128:(j + 1) * 128],
                                    yp_t[64 * h2:64 * h2 + 64,
                                         nh * nw:(nh + 1) * nw],
                                    start=True, stop=True,
                                    tile_position=(64 * h2, 0),
                                )
                        for h2, want_act, a_t, ob_t in gs:
                            if want_act:
                                nc.scalar.activation(
                                    a_t[:, ji * 1024:(ji + 1) * 1024],
                                    psjs[h2][:],
                                    AF.Copy, bias=bias_a, scale=s_const)
                                if P2_HALVES:
                                    lo, hi = ji * 1024, (ji + 1) * 1024

                                    def fmap(a_t=a_t, ob_t=ob_t,
                                             lo=lo, hi=hi):
                                        nc.gpsimd.tensor_scalar(
                                            ob_t[:, lo:hi], a_t[:, lo:hi],
                                            c_b, d_b, OP.mult, OP.add)
                                    pending.append(fmap)
                            else:
                                nc.vector._custom_dve(
                                    TRUNC_OP,
                                    out=ob_t[:, ji * 1024:(ji + 1) * 1024],
                                    in0=psjs[h2][:], s0=s2c, s1=b2c,
                                    imm2=0.5)
                    for h2, want_act, a_t, ob_t in gs:
                        if want_act and not P2_HALVES:
                            if (act_g - 1) % 25 < P2_ACT_OF:
                                def fmap(a_t=a_t, ob_t=ob_t):
                                    nc.scalar.activation(
                                        ob_t[:], a_t[:], AF.Copy,
                                        bias=d_b, scale=c_b)
                            else:
                                def fmap(a_t=a_t, ob_t=ob_t):
                                    nc.gpsimd.tensor_scalar(
                                        ob_t[:], a_t[:], c_b, d_b,
                                        OP.mult, OP.add)
                            pending.append(fmap)
                    while len(pending) > DEFER_N:
                        pending.pop(0)()
                    continue
                for h2 in range(2):
                    # per-group engine split: ACT two-pass (+Pool pass2) vs
                    # fused single-pass custom DVE op straight to int8
                    want_act = ((gidx + 1) * ACT_G_OF_48) // 48 > act_g
                    gidx += 1
                    if want_act:
                        act_g += 1
                    need_a = (SPLIT_JI or want_act) and not BENCH_NO_P1
                    a_cols = 1024 if SPLIT_JI else 2048
                    a_t = (apool.tile([128, a_cols], mybir.dt.int16,
                                      tag="a", name=f"a_{pair}_{h2}_{jg}")
                           if need_a else None)
                    if h2 not in ob:
                        ob[h2] = opool.tile([128, 8192], mybir.dt.int8,
                                            tag="obs",
                                            name=f"ob_{pair}_{h2}")
                        if BENCH_NO_P1 or BENCH_NO_P2:
                            # bench-only: ensure the store has a writer even
                            # when the producing ops are ablated
                            nc.vector.memset(ob[h2][:, 0:8192], 0)
                    ob_t = ob[h2][:, jg * 2048:(jg + 1) * 2048]
                    for ji in range(2):
                        j = jg * 2 + ji
                        lhsT = xt_t[64 * h2:64 * h2 + 64,
                                    j * 128:(j + 1) * 128]
                        psj = (ps_dummy if BENCH_NO_MM else
                               pspool.tile([128, 1024], mybir.dt.float32,
                                           tag="ps",
                                           name=f"ps_{pair}_{h2}_{jg}_{ji}"))
                        for nh in (() if BENCH_NO_MM else range(NH_SPLIT)):
                            nw = 1024 // NH_SPLIT
                            nc.tensor.matmul(
                                psj[:, nh * nw:(nh + 1) * nw],
                                lhsT,
                                yp_t[64 * h2:64 * h2 + 64,
                                     nh * nw:(nh + 1) * nw],
                                start=True, stop=True,
                                tile_position=(64 * h2, 0),
                            )
                        # drain this quad now; 4-buffer rotation keeps the
                        # fills decoupled
                        quad_act = (ji == 0) if SPLIT_JI else want_act
                        if BENCH_NO_P1:
                            pass
                        elif quad_act:
                            a_dst = (a_t[:, 0:1024] if SPLIT_JI else
                                     a_t[:, ji * 1024:(ji + 1) * 1024])
                            nc.scalar.activation(
                                a_dst, psj[:],
                                AF.Copy, bias=bias_a, scale=s_const)
                        else:
                            nc.vector._custom_dve(
                                TRUNC_OP,
                                out=ob_t[:, ji * 1024:(ji + 1) * 1024],
                                in0=psj[:], s0=s2c, s1=b2c, imm2=0.5)
                    # Pool pass2 for the ACT-drained columns, deferred so
                    # it trails pass1 by DEFER_N groups
                    if (SPLIT_JI or want_act) and not BENCH_NO_P1 \
                            and not BENCH_NO_P2:
                        p2_dst = ob_t[:, 0:1024] if SPLIT_JI else ob_t[:]
                        on_dve = (act_g - 1) % 25 < P2_DVE_OF \
                            if want_act else False
                        if on_dve:
                            def fmap(a_t=a_t, p2_dst=p2_dst):
                                nc.vector._custom_dve(
                                    P2_OP, out=p2_dst, in0=a_t[:])
                        else:
                            def fmap(a_t=a_t, p2_dst=p2_dst):
                                nc.gpsimd.tensor_scalar(p2_dst, a_t[:],
                                                        c_b, d_b,
                                                        OP.mult, OP.add)
                        pending.append(fmap)
                    while len(pending) > DEFER_N:
                        pending.pop(0)()
            # flush this pair's remaining pass2 ops, then batched output DMAs
            while pending:
                pending.pop(0)()
            for h2 in range(2):
                if BENCH_NO_STORE:
                    continue
                dst = d_o[2 * pair + h2, :, :].rearrange(
                    "(p j) t -> p (j t)", j=M_BLOCKS)
                dma(dst[:, :], ob[h2][:, 0:8192])


def default_key():
    """Requant constants for the reference problem's quantization params -
    used by bench.py when kernel() hasn't run in this process."""
    s_const = float(np.float32(np.float32(0.000234) / np.float32(0.0625)))
    bias_a = float(np.float64(np.float32(2.0)) - 0.5 + 2.0 ** -18)
    c_b = float(np.float32(255.0 / 256.0))
    d_b = float(np.float32(0.499))
    return (s_const, bias_a, c_b, d_b)


def kernel(x, y, alpha, a_zp, b_zp, out_zp, o_alpha):
    global LAST_RESULTS, LAST_PREP
    x = np.asarray(x)
    y = np.asarray(y)
    s_const = float(np.float32(np.float32(alpha) / np.float32(o_alpha)))
    bias_a = float(np.float64(np.float32(out_zp)) - 0.5 + 2.0 ** -18)
    c_b = float(np.float32(255.0 / 256.0))
    d_b = float(np.float32(0.499))

    # ---- host-side shard + dequant prep (exact in bf16) ----
    xf = x.reshape(B * H, S, D).astype(np.float32) - np.float32(a_zp)
    yf = y.reshape(B * H, D, T).astype(np.float32) - np.float32(b_zp)
    # lhsT layout: [head, D, S], head-pairs stacked to 128 partitions.
    # S-columns permuted to c = j*128 + p  <->  s = 8p + j so each psum
    # partition owns 8 consecutive output rows (8 KiB DMA runs).
    xt = np.ascontiguousarray(xf.transpose(0, 2, 1)).astype(ml_dtypes.bfloat16)
    xt = np.ascontiguousarray(
        xt.reshape(B * H, D, S // 8, 8).transpose(0, 1, 3, 2)).reshape(
        B * H, D, S)
    yp = yf.astype(ml_dtypes.bfloat16)
    xt = xt.reshape(N_CORES, N_PAIRS, 128, S)
    yp = yp.reshape(N_CORES, N_PAIRS, 128, T)
    # pack [xt | yp] so each pair is one input dma_start on-device
    xy = np.concatenate([xt, yp], axis=-1)

    key = (s_const, bias_a, c_b, d_b)
    if key not in _NC_CACHE:
        _NC_CACHE[key] = _build_core_program(*key)
    nc = _NC_CACHE[key]

    in_maps = [{"xy": xy[c]} for c in range(N_CORES)]
    LAST_PREP = (key, in_maps)
    res = run_bass_kernel_spmd(nc, in_maps, core_ids=list(range(N_CORES)))
    LAST_RESULTS = res

    out = np.stack([res.results[c]["o"] for c in range(N_CORES)])
    return out.reshape(B, H, S, T)


if __name__ == "__main__":
    rng = np.random.default_rng(0)
    x = rng.integers(-128, 128, size=(B, H, S, D)).astype(np.int32)
    y = rng.integers(-128, 128, size=(B, H, D, T)).astype(np.int32)
    out = kernel(x=x, y=y, alpha=np.float32(0.000234), a_zp=np.float32(3.0),
                 b_zp=np.float32(-5.0), out_zp=np.float32(2.0),
                 o_alpha=np.float32(0.0625))
    print("kernel output", out.shape, out.dtype)



# revision 45
# speedup vs baseline: 1.1279x; 1.1279x over previous
"""Trainium2 Bass kernel for nn_BMMS8TS8NS8T: batched int8-valued GEMM with
dequant/requant, sharded head-parallel across 8 NeuronCores.

Reference semantics (jax CPU, fp32):
    a = x.float() - a_zp          # [B,H,S,D]  int8-valued
    b = y.float() - b_zp          # [B,H,D,T]
    q = a @ b                     # exact integers (|q| <= 64*131*132 < 2^24)
    v = fl(fl(q * s) + out_zp),   s = fl(alpha / o_alpha)
    out = trunc(clip(v, -128, 127)).astype(int8)   # trunc toward zero

Device strategy per core (12 heads = (B*H)/8, head parallel, no cross-core
communication):
  - host pre-dequantizes inputs to bf16 (exact: integers with |v| <= 132 are
    exactly representable) and pre-transposes/permutes x so the stationary
    matmul operand needs no on-device transpose; x^T and y are packed into
    ONE dram tensor per head-pair so each pair costs one input dma_start
  - TensorE: K=64 matmuls, two heads packed in the 128-row PE array via row
    tiling (tile_position (0,0)/(64,0)); fp32 PSUM accumulation is exact.
    The two heads' matmuls are emitted INTERLEAVED (H2_ILV) so adjacent PE
    instructions land on different row-groups and execute concurrently
    (~2x effective PE rate, earlier drain starts; measured -4us/iter)
  - PSUM as 4 rotating [128,1024] quads (bufs=4) so matmul fills decouple
    from drains (2x[128,2048] ping-pong was period-bound: a tile's refill
    waited on its own ~2.1us drain -> ~71us/iter cap)
  - requantization: the key insight is a CUSTOM DVE uop (per-NEFF uop table,
    registered at import into concourse.dve_ops) that does the whole
    requant+trunc in ONE 1x pass from PSUM fp32 straight to int8:
        u   = fl(fl(q * s2) + zp),  s2 = fl(s * (1 + 2^-22))
        out = sat_i8(RNE(u + 0.5*((u < -0.5) - (u > 0.5))))
    The three-zone +-0.5 shift makes RNE produce trunc-toward-zero incl.
    the double-width bin at 0 that no single affine+RNE can express; the
    2^-22 scale inflation breaks RNE half-to-even ties symmetrically
    (correct for trunc on both signs).  Validated exhaustively on host over
    every reachable q: 0 mismatches (saturation included); hardware runs
    bit-exact vs the jax reference.
  - 1x PSUM-read capacity is the structural floor (GpSimd has NO PSUM port;
    fp32 PSUM reads are 1x on ScalarE/VectorE -> >= 12.6M reads /
    276G elem/s ~= 46us/iter minimum).  To use all three post-PE engines,
    ACT_G_OF_48 groups take a two-pass path (ScalarE pass1 -> int16,
    GpSimd/Pool pass2 tensor_scalar A*(255/256)+0.499) and the rest use the
    fused VectorE op.  Measured balance ~51/57/45us (ACT/DVE/Pool) per
    iteration; Pool ops stretch under DVE shared-SBUF-port contention, so
    pushing pass2 work to DVE (STT or a 2x custom op) measured WORSE.
  - x^T columns are host-permuted so psum partition p owns output rows
    s = 8p+j: a whole head's output is one [128, 8192] int8 staging tile
    stored with a single dma_start (dma_start costs ~0.6-4us of sequencer
    descriptor-generation time; 60 small DMAs saturated the SP ring, and
    alternating rings (DMA_ALT) stalls ScalarE compute - keep all on SP)
Loop-slope A/B (hardware, same-process interleaved, min-of-reps):
    v1 two-pass split-engine baseline:    96-102 us/iter (graded 548503 ns)
    + 4-quad psum, Pool pass2, fused op:  60-76 us/iter
    + h2-interleaved matmul emission:     ~1.06x further (final 64.5 us/iter
      with exact output in the same test.py run; absolute scale drifts
      ~25% between runs - within-run ratios are the trusted signal)
"""

from contextlib import ExitStack
import numpy as np
import ml_dtypes

import concourse.bacc as bacc
import concourse.tile as tile
from concourse import mybir
from concourse import dve_ops as _dve_ops
from concourse.bass_utils import run_bass_kernel_spmd
from concourse.dve_spec import C0, C1, C2, Spec, Src0, Zero, lower
from concourse.dve_uop import DveOpSpec

AF = mybir.ActivationFunctionType
OP = mybir.AluOpType
BF16 = mybir.dt.bfloat16


def _trunc_requant_ref(in0, in1, s0, s1, imm2):
    """Faithful fp32 emulation of TRUNC_REQUANT_ANT for CoreSim."""
    f32 = np.float32
    u = (in0.astype(f32) * f32(s0)).astype(f32) + f32(s1)
    u = u.astype(f32)
    ind = (u < f32(-imm2)).astype(f32) - (u > f32(imm2)).astype(f32)
    return (u + (f32(imm2) * ind).astype(f32)).astype(f32)


def _register_trunc_op():
    """Custom DVE uop: single-pass exact requant+trunc, psum fp32 -> int8.

        u   = fl(fl(q * s2) + zp)               (s2 = s * (1 + 2^-22))
        out = sat_i8(RNE(u + 0.5*((u < -0.5) - (u > 0.5))))

    The three-zone correction shifts u by +-0.5 so RNE lands on
    trunc-toward-zero; the middle zone gives the double-width output bin at
    0 that no single affine+RNE can produce.  Validated exhaustively on the
    host over every reachable q in [-1106688, 1047552]: 0 mismatches vs the
    reference fp32 chain (including saturation; RNE half-to-even ties fixed
    by the 2^-22 scale inflation, symmetric for trunc).
    """
    name = "TRUNC_REQUANT_ANT"
    for op in _dve_ops.OPS:
        if op.name == name:
            return op
    _u = Src0 * C0 + C1
    spec = Spec(
        body=_u + C2 * ((_u < -C2) - (_u > C2)),
        reference=_trunc_requant_ref,
    )
    row = max(_dve_ops._SUB_OPCODE_FOR_NAME.values()) + 1
    assert row < 0x20
    shas = {}
    for ver in ("v3", "v4"):
        try:
            uops = lower(spec, ver=ver)
            shas[ver] = DveOpSpec(name=name, opcode=row, uops=uops,
                                  rd1_en=False).sha(ver)
        except Exception:
            pass
    op = _dve_ops.DveOp(name=name, spec=spec, subdim=False, uops_sha=shas)
    _dve_ops.OPS.append(op)
    _dve_ops.CUSTOM_DVE_SPECS[name] = spec
    _dve_ops._SUB_OPCODE_FOR_NAME[name] = row
    return op


TRUNC_OP = _register_trunc_op()


def _pass2_ref(in0, in1, s0, s1, imm2):
    f32 = np.float32
    a = in0.astype(f32)
    return (a + (a < f32(0.0)).astype(f32)).astype(f32)


def _register_pass2_op():
    """Custom DVE uop for pass2: out_i8 = sat(RNE(A + [A<0])), A int16.
    Single-src, 16-bit -> eligible for the 2x_1P perf slot (perf_en), which
    doesn't touch the DVE/Pool shared SBUF port pair."""
    name = "P2_TRUNC_ANT"
    for op in _dve_ops.OPS:
        if op.name == name:
            return op
    spec = Spec(body=Src0 + (Src0 < Zero), reference=_pass2_ref)
    row = max(_dve_ops._SUB_OPCODE_FOR_NAME.values()) + 1
    assert row < 0x20
    shas = {}
    perf = {}
    for ver in ("v3", "v4"):
        try:
            uops = lower(spec, ver=ver)
            shas[ver] = DveOpSpec(name=name, opcode=row, uops=uops,
                                  rd1_en=False).sha(ver)
            perf[ver] = True
        except Exception:
            pass
    op = _dve_ops.DveOp(name=name, spec=spec, subdim=False, uops_sha=shas,
                        perf_en=perf)
    _dve_ops.OPS.append(op)
    _dve_ops.CUSTOM_DVE_SPECS[name] = spec
    _dve_ops._SUB_OPCODE_FOR_NAME[name] = row
    return op


P2_OP = _register_pass2_op()

N_CORES = 8
B, H, S, D = 8, 12, 1024, 64
HEADS_PER_CORE = B * H // N_CORES          # 12
N_PAIRS = HEADS_PER_CORE // 2              # 6
M_BLOCKS = S // 128                        # 8
T = 1024

# set by kernel() for test.py / bench.py to inspect
LAST_RESULTS = None
LAST_PREP = None

_NC_CACHE = {}

# --- engine-assignment knobs (all values hardware-A/B'd; see docstring)
H2_ILV = True           # interleave h0/h1 matmul emission for PE row-group
                        # concurrency (PE 41us -> ~21us busy; also removes
                        # any HAM rethrottle risk from PE idle gaps)
P2_DVE_OF = 0           # ACT-group pass2 ops moved to the DVE 2x custom op
                        # (measured monotonically WORSE: 4->+21us, 25->+46us
                        # per iter - DVE is the co-busiest engine)
SPLIT_JI = False        # per-group quad split ji0->ACT+Pool / ji1->fused
                        # DVE (measured +19us/iter worse: every group having
                        # adjacent DVE+Pool ops maximizes the shared-port
                        # contention stretch on Pool)
ACT_G_OF_48 = 25        # of the 48 groups, how many take the ScalarE
                        # two-pass path (ACT pass1 int16 + Pool pass2);
                        # the rest use the fused single-pass custom DVE op
                        # (TRUNC_REQUANT_ANT, psum fp32 -> int8 directly)
A_BUFS = 6              # [128, 2048] int16 pass1 staging tiles
ACT_BY_H2 = False       # group->engine by head instead of Bresenham:
                        # h0 -> ACT two-pass, h1 -> fused DVE (24/24) so the
                        # two concurrent drains of every j-block are always
                        # on different engines
P2_ACT_OF = 0           # of the ACT-groups' pass2 ops, how many run on
                        # ScalarE (activation Copy(A*c_b + d_b), identical
                        # numerics) instead of Pool - trades stretched Pool
                        # time (~2.9us/op under DVE port contention) for ACT
                        # slack (~2.0us/op)
P2_HALVES = False       # Pool pass2 as two [128,1024] ops per group (can
                        # start after the first ACT quad-drain; finer Pool
                        # queue granularity) vs one [128,2048] op
OB_BUFS = 4             # [128, 8192] int8 staging tiles (one per head;
                        # 4th buffer relieves a store-WAR stall under the
                        # interleaved emission - measured -4.7us/iter)
XIN_BUFS = 2            # packed [128, 2048] bf16 input tiles
DMA_ALT = False         # alternate dma_start between the two HWDGE rings
                        # (SP qSPDynamicHW / ACT qActDynamicHW) - descriptor
                        # generation is ~1.7-4us of sequencer time per DMA
                        # and saturates a single ring
DEFER_N = 2             # how many groups the pass2 trails its pass1
NH_SPLIT = 2            # rhs blocks per ji (N=512 moving operand)
SPLIT_FIRST_LOAD = True

# bench-only ablation knobs (correctness-invalid when set; used by ab.py to
# attribute loop-slope time to pipeline stages)
BENCH_NO_STORE = False
BENCH_NO_LOAD = False
BENCH_NO_MM = False
BENCH_NO_P2 = False
BENCH_NO_P1 = False


def _build_core_program(s_const: float, bias_a: float, c_b: float, d_b: float,
                        loop_iters: int | None = None,
                        bench_io: bool = False):
    """One NeuronCore's program: 12 heads of [1024,64]@[64,1024] + requant.

    loop_iters: when set, wraps the whole body in a hardware For_i loop -
    used only for benchmarking (device time scales with the loop count so a
    slope isolates HW exec time from host/relay dispatch overhead).

    bench_io: all big DRAM tensors become Internal (device-resident garbage,
    never shipped over the axon relay) and a tiny [1,16] ExternalOutput is
    added so PJRT has something to return.  Timing is data-independent, so
    the loop slope is unaffected; per-call payload drops from ~15 MB/core to
    16 bytes.
    """
    nc = bacc.Bacc("TRN2", target_bir_lowering=False, debug=False)
    io_kind_in = "Internal" if bench_io else "ExternalInput"
    io_kind_out = "Internal" if bench_io else "ExternalOutput"
    # head-pairs stacked on the partition axis; xt and yp packed into one
    # tensor so each pair needs a single input dma_start (SP sequencer DGE
    # config is ~1.7us per dma_start - the dominant serial cost at 60 DMAs)
    d_xy = nc.dram_tensor("xy", [N_PAIRS, 128, S + T], BF16, kind=io_kind_in)
    d_o = nc.dram_tensor("o", [HEADS_PER_CORE, S, T], mybir.dt.int8,
                         kind=io_kind_out)
    d_tick = (nc.dram_tensor("tick", [1, 16], mybir.dt.int8,
                             kind="ExternalOutput") if bench_io else None)

    with tile.TileContext(nc) as tc:
        with ExitStack() as stk:
            if loop_iters is not None:
                # PE's body exceeds one IRAM block; hint the back-edge so the
                # benchmark loop doesn't pay a ~3-4 us ifetch per iteration
                # that single-shot execution would not pay.
                stk.enter_context(tc.For_i(0, loop_iters, 1,
                                           hint_engines=(mybir.EngineType.PE,)))
            _emit_body(nc, tc, d_xy, d_o, s_const, bias_a, c_b, d_b)
        if d_tick is not None:
            with tc.tile_pool(name="tick", bufs=1) as tkpool:
                tk = tkpool.tile([1, 16], mybir.dt.int8, tag="tick")
                nc.vector.memset(tk[:], 0)
                nc.sync.dma_start(d_tick[:, :], tk[:])
    nc.compile()
    return nc


def _emit_body(nc, tc, d_xy, d_o, s_const, bias_a, c_b, d_b):
    # VectorE one-op pass1 constants (validated in validate_requant.py over
    # every reachable q: 0 mismatches, max tie margin variant)
    s64 = np.float64(np.float32(s_const))
    zp64 = np.float64(np.float32(bias_a)) + 0.5 - 2.0 ** -18  # recover out_zp
    b0 = float(np.float32((zp64 - 0.5) / s64 + 2.0 ** -8))
    s2 = float(np.float32(s64 * (1.0 + 2.0 ** -22)))

    # fused-op constants (exhaustively validated, see _register_trunc_op)
    s2c = float(np.float32(np.float64(np.float32(s_const)) *
                           (1.0 + 2.0 ** -22)))
    b2c = float(np.float32(np.float64(np.float32(bias_a)) + 0.5 - 2.0 ** -18))

    gidx = 0            # group counter (0..47)
    act_g = 0
    pending = []
    dma_i = [0]

    def dma(dst, src):
        # alternate between the two HWDGE rings; Tile still tracks deps
        eng = nc.sync if (not DMA_ALT or dma_i[0] % 2 == 0) else nc.scalar
        dma_i[0] += 1
        eng.dma_start(dst, src)
    with tc.tile_pool(name="xin", bufs=XIN_BUFS) as xpool, \
         tc.tile_pool(name="aint", bufs=A_BUFS) as apool, \
         tc.tile_pool(name="obuf", bufs=OB_BUFS) as opool, \
         tc.tile_pool(name="ps", bufs=4, space="PSUM") as pspool:
        tiles = [None] * N_PAIRS
        xy0 = xpool.tile([128, S + T], BF16, tag="xy")
        if BENCH_NO_LOAD:
            # bench-only: no HBM input traffic; one memset tile feeds every
            # head-pair (DVE ~0.9us/iter pollution, noted in readings)
            nc.vector.memset(xy0[:], 0)
        elif SPLIT_FIRST_LOAD:
            # single-shot prologue: land group-0's operands first so the
            # first matmuls start earlier (xt cols 0:256 + all of yp)
            dma(xy0[:, 0:256], d_xy[0, :, 0:256])
            dma(xy0[:, S:S + T], d_xy[0, :, S:S + T])
            dma(xy0[:, 256:S], d_xy[0, :, 256:S])
        else:
            dma(xy0[:], d_xy[0, :, :])
        tiles[0] = xy0

        ps_dummy = None
        if BENCH_NO_MM:
            # bench-only: pass1 reads this one pre-written quad (keeps the
            # psum-read cost while removing the matmul fills)
            ps_dummy = pspool.tile([128, 1024], mybir.dt.float32, tag="ps")
            nc.vector.memset(ps_dummy[:], 0)

        for pair in range(N_PAIRS):
            xy_t = tiles[pair]
            xt_t = xy_t[:, 0:S]
            yp_t = xy_t[:, S:S + T]
            if pair + 1 < N_PAIRS:
                # prefetch next pair's operands now so the loads sit ahead
                # of this pair's output stores in the SP HWDGE FIFO
                if BENCH_NO_LOAD:
                    tiles[pair + 1] = xy0
                else:
                    xy_n = xpool.tile([128, S + T], BF16, tag="xy")
                    dma(xy_n[:], d_xy[pair + 1, :, :])
                    tiles[pair + 1] = xy_n

            ob = {}

            for jg in range(M_BLOCKS // 2):
                if H2_ILV:
                    # interleave the two heads' matmuls so adjacent PE
                    # instructions target different row-groups (0,0)/(64,0)
                    # and execute concurrently (~2x effective PE rate);
                    # drains per quad follow right after its two matmuls.
                    gs = []
                    for h2 in range(2):
                        if ACT_BY_H2:
                            # strict per-j-block engine pairing (h0->ACT,
                            # h1->DVE) keeping the 25/23 ratio: one extra
                            # ACT group mid-iteration (group 25 -> h1 slot)
                            want_act = (h2 == 0) or (gidx == 25)
                            gidx += 1
                            if want_act:
                                act_g += 1
                        else:
                            want_act = ((gidx + 1) * ACT_G_OF_48) // 48 \
                                > act_g
                            gidx += 1
                            if want_act:
                                act_g += 1
                        a_t = (apool.tile([128, 2048], mybir.dt.int16,
                                          tag="a",
                                          name=f"a_{pair}_{h2}_{jg}")
                               if want_act else None)
                        if h2 not in ob:
                            ob[h2] = opool.tile([128, 8192], mybir.dt.int8,
                                                tag="obs",
                                                name=f"ob_{pair}_{h2}")
                        ob_t = ob[h2][:, jg * 2048:(jg + 1) * 2048]
                        gs.append((h2, want_act, a_t, ob_t))
                    for ji in range(2):
                        j = jg * 2 + ji
                        psjs = {}
                        for h2, _, _, _ in gs:
                            psjs[h2] = pspool.tile(
                                [128, 1024], mybir.dt.float32, tag="ps",
                                name=f"ps_{pair}_{h2}_{jg}_{ji}")
                        for nh in range(NH_SPLIT):
                            nw = 1024 // NH_SPLIT
                            for h2, _, _, _ in gs:
                                nc.tensor.matmul(
                                    psjs[h2][:, nh * nw:(nh + 1) * nw],
                                    xt_t[64 * h2:64 * h2 + 64,
                                         j * 128:(j + 1) * 128],
                                    yp_t[64 * h2:64 * h2 + 64,
                                         nh * nw:(nh + 1) * nw],
                                    start=True, stop=True,
                                    tile_position=(64 * h2, 0),
                                )
                        for h2, want_act, a_t, ob_t in gs:
                            if want_act:
                                nc.scalar.activation(
                                    a_t[:, ji * 1024:(ji + 1) * 1024],
                                    psjs[h2][:],
                                    AF.Copy, bias=bias_a, scale=s_const)
                                if P2_HALVES:
                                    lo, hi = ji * 1024, (ji + 1) * 1024

                                    def fmap(a_t=a_t, ob_t=ob_t,
                                             lo=lo, hi=hi):
                                        nc.gpsimd.tensor_scalar(
                                            ob_t[:, lo:hi], a_t[:, lo:hi],
                                            c_b, d_b, OP.mult, OP.add)
                                    pending.append(fmap)
                            else:
                                nc.vector._custom_dve(
                                    TRUNC_OP,
                                    out=ob_t[:, ji * 1024:(ji + 1) * 1024],
                                    in0=psjs[h2][:], s0=s2c, s1=b2c,
                                    imm2=0.5)
                    for h2, want_act, a_t, ob_t in gs:
                        if want_act and not P2_HALVES:
                            if (act_g - 1) % 25 < P2_ACT_OF:
                                def fmap(a_t=a_t, ob_t=ob_t):
                                    nc.scalar.activation(
                                        ob_t[:], a_t[:], AF.Copy,
                                        bias=d_b, scale=c_b)
                            else:
                                def fmap(a_t=a_t, ob_t=ob_t):
                                    nc.gpsimd.tensor_scalar(
                                        ob_t[:], a_t[:], c_b, d_b,
                                        OP.mult, OP.add)
                            pending.append(fmap)
                    while len(pending) > DEFER_N:
                        pending.pop(0)()
                    continue
                for h2 in range(2):
                    # per-group engine split: ACT two-pass (+Pool pass2) vs
                    # fused single-pass custom DVE op straight to int8
                    want_act = ((gidx + 1) * ACT_G_OF_48) // 48 > act_g
                    gidx += 1
                    if want_act:
                        act_g += 1
                    need_a = (SPLIT_JI or want_act) and not BENCH_NO_P1
                    a_cols = 1024 if SPLIT_JI else 2048
                    a_t = (apool.tile([128, a_cols], mybir.dt.int16,
                                      tag="a", name=f"a_{pair}_{h2}_{jg}")
                           if need_a else None)
                    if h2 not in ob:
                        ob[h2] = opool.tile([128, 8192], mybir.dt.int8,
                                            tag="obs",
                                            name=f"ob_{pair}_{h2}")
                        if BENCH_NO_P1 or BENCH_NO_P2:
                            # bench-only: ensure the store has a writer even
                            # when the producing ops are ablated
                            nc.vector.memset(ob[h2][:, 0:8192], 0)
                    ob_t = ob[h2][:, jg * 2048:(jg + 1) * 2048]
                    for ji in range(2):
                        j = jg * 2 + ji
                        lhsT = xt_t[64 * h2:64 * h2 + 64,
                                    j * 128:(j + 1) * 128]
                        psj = (ps_dummy if BENCH_NO_MM else
                               pspool.tile([128, 1024], mybir.dt.float32,
                                           tag="ps",
                                           name=f"ps_{pair}_{h2}_{jg}_{ji}"))
                        for nh in (() if BENCH_NO_MM else range(NH_SPLIT)):
                            nw = 1024 // NH_SPLIT
                            nc.tensor.matmul(
                                psj[:, nh * nw:(nh + 1) * nw],
                                lhsT,
                                yp_t[64 * h2:64 * h2 + 64,
                                     nh * nw:(nh + 1) * nw],
                                start=True, stop=True,
                                tile_position=(64 * h2, 0),
                            )
                        # drain this quad now; 4-buffer rotation keeps the
                        # fills decoupled
                        quad_act = (ji == 0) if SPLIT_JI else want_act
                        if BENCH_NO_P1:
                            pass
                        elif quad_act:
                            a_dst = (a_t[:, 0:1024] if SPLIT_JI else
                                     a_t[:, ji * 1024:(ji + 1) * 1024])
                            nc.scalar.activation(
                                a_dst, psj[:],
                                AF.Copy, bias=bias_a, scale=s_const)
                        else:
                            nc.vector._custom_dve(
                                TRUNC_OP,
                                out=ob_t[:, ji * 1024:(ji + 1) * 1024],
                                in0=psj[:], s0=s2c, s1=b2c, imm2=0.5)
                    # Pool pass2 for the ACT-drained columns, deferred so
                    # it trails pass1 by DEFER_N groups
                    if (SPLIT_JI or want_act) and not BENCH_NO_P1 \
                            and not BENCH_NO_P2:
                        p2_dst = ob_t[:, 0:1024] if SPLIT_JI else ob_t[:]
                        on_dve = (act_g - 1) % 25 < P2_DVE_OF \
                            if want_act else False
                        if on_dve:
                            def fmap(a_t=a_t, p2_dst=p2_dst):
                                nc.vector._custom_dve(
                                    P2_OP, out=p2_dst, in0=a_t[:])
                        else:
                            def fmap(a_t=a_t, p2_dst=p2_dst):
                                nc.gpsimd.tensor_scalar(p2_dst, a_t[:],
                                                        c_b, d_b,
                                                        OP.mult, OP.add)
                        pending.append(fmap)
                    while len(pending) > DEFER_N:
                        pending.pop(0)()
            # flush this pair's remaining pass2 ops, then batched output DMAs
            while pending:
                pending.pop(0)()
            for h2 in range(2):
                if BENCH_NO_STORE:
                    continue
                dst = d_o[2 * pair + h2, :, :].rearrange(
                    "(p j) t -> p (j t)", j=M_BLOCKS)
                dma(dst[:, :], ob[h2][:, 0:8192])


def default_key():
    """Requant constants for the reference problem's quantization params -
    used by bench.py when kernel() hasn't run in this process."""
    s_const = float(np.float32(np.float32(0.000234) / np.float32(0.0625)))
    bias_a = float(np.float64(np.float32(2.0)) - 0.5 + 2.0 ** -18)
    c_b = float(np.float32(255.0 / 256.0))
    d_b = float(np.float32(0.499))
    return (s_const, bias_a, c_b, d_b)


def kernel(x, y, alpha, a_zp, b_zp, out_zp, o_alpha):
    global LAST_RESULTS, LAST_PREP
    x = np.asarray(x)
    y = np.asarray(y)
    s_const = float(np.float32(np.float32(alpha) / np.float32(o_alpha)))
    bias_a = float(np.float64(np.float32(out_zp)) - 0.5 + 2.0 ** -18)
    c_b = float(np.float32(255.0 / 256.0))
    d_b = float(np.float32(0.499))

    # ---- host-side shard + dequant prep (exact in bf16) ----
    xf = x.reshape(B * H, S, D).astype(np.float32) - np.float32(a_zp)
    yf = y.reshape(B * H, D, T).astype(np.float32) - np.float32(b_zp)
    # lhsT layout: [head, D, S], head-pairs stacked to 128 partitions.
    # S-columns permuted to c = j*128 + p  <->  s = 8p + j so each psum
    # partition owns 8 consecutive output rows (8 KiB DMA runs).
    xt = np.ascontiguousarray(xf.transpose(0, 2, 1)).astype(ml_dtypes.bfloat16)
    xt = np.ascontiguousarray(
        xt.reshape(B * H, D, S // 8, 8).transpose(0, 1, 3, 2)).reshape(
        B * H, D, S)
    yp = yf.astype(ml_dtypes.bfloat16)
    xt = xt.reshape(N_CORES, N_PAIRS, 128, S)
    yp = yp.reshape(N_CORES, N_PAIRS, 128, T)
    # pack [xt | yp] so each pair is one input dma_start on-device
    xy = np.concatenate([xt, yp], axis=-1)

    key = (s_const, bias_a, c_b, d_b)
    if key not in _NC_CACHE:
        _NC_CACHE[key] = _build_core_program(*key)
    nc = _NC_CACHE[key]

    in_maps = [{"xy": xy[c]} for c in range(N_CORES)]
    LAST_PREP = (key, in_maps)
    res = run_bass_kernel_spmd(nc, in_maps, core_ids=list(range(N_CORES)))
    LAST_RESULTS = res

    out = np.stack([res.results[c]["o"] for c in range(N_CORES)])
    return out.reshape(B, H, S, T)


if __name__ == "__main__":
    rng = np.random.default_rng(0)
    x = rng.integers(-128, 128, size=(B, H, S, D)).astype(np.int32)
    y = rng.integers(-128, 128, size=(B, H, D, T)).astype(np.int32)
    out = kernel(x=x, y=y, alpha=np.float32(0.000234), a_zp=np.float32(3.0),
                 b_zp=np.float32(-5.0), out_zp=np.float32(2.0),
                 o_alpha=np.float32(0.0625))
    print("kernel output", out.shape, out.dtype)



# revision 47
# speedup vs baseline: 1.2120x; 1.0746x over previous
"""Trainium2 Bass kernel for nn_BMMS8TS8NS8T: batched int8-valued GEMM with
dequant/requant, sharded head-parallel across 8 NeuronCores.

Reference semantics (jax CPU, fp32):
    a = x.float() - a_zp          # [B,H,S,D]  int8-valued
    b = y.float() - b_zp          # [B,H,D,T]
    q = a @ b                     # exact integers (|q| <= 64*131*132 < 2^24)
    v = fl(fl(q * s) + out_zp),   s = fl(alpha / o_alpha)
    out = trunc(clip(v, -128, 127)).astype(int8)   # trunc toward zero

Device strategy per core (12 heads = (B*H)/8, head parallel, no cross-core
communication):
  - host pre-dequantizes inputs to bf16 (exact: integers with |v| <= 132 are
    exactly representable) and pre-transposes/permutes x so the stationary
    matmul operand needs no on-device transpose; x^T and y are packed into
    ONE dram tensor per head-pair so each pair costs one input dma_start
  - TensorE: K=64 matmuls, two heads packed in the 128-row PE array via row
    tiling (tile_position (0,0)/(64,0)); fp32 PSUM accumulation is exact.
    The two heads' matmuls are emitted INTERLEAVED (H2_ILV) so adjacent PE
    instructions land on different row-groups and execute concurrently
    (~2x effective PE rate, earlier drain starts; measured -4us/iter)
  - PSUM as 4 rotating [128,1024] quads (bufs=4) so matmul fills decouple
    from drains (2x[128,2048] ping-pong was period-bound: a tile's refill
    waited on its own ~2.1us drain -> ~71us/iter cap)
  - requantization: the key insight is a CUSTOM DVE uop (per-NEFF uop table,
    registered at import into concourse.dve_ops) that does the whole
    requant+trunc in ONE 1x pass from PSUM fp32 straight to int8:
        u   = fl(fl(q * s2) + zp),  s2 = fl(s * (1 + 2^-22))
        out = sat_i8(RNE(u + 0.5*((u < -0.5) - (u > 0.5))))
    The three-zone +-0.5 shift makes RNE produce trunc-toward-zero incl.
    the double-width bin at 0 that no single affine+RNE can express; the
    2^-22 scale inflation breaks RNE half-to-even ties symmetrically
    (correct for trunc on both signs).  Validated exhaustively on host over
    every reachable q: 0 mismatches (saturation included); hardware runs
    bit-exact vs the jax reference.
  - 1x PSUM-read capacity is the structural floor (GpSimd has NO PSUM port;
    fp32 PSUM reads are 1x on ScalarE/VectorE -> >= 12.6M reads /
    276G elem/s ~= 46us/iter minimum).  To use all three post-PE engines,
    ACT_G_OF_48 groups take a two-pass path (ScalarE pass1 -> int16,
    GpSimd/Pool pass2 tensor_scalar A*(255/256)+0.499) and the rest use the
    fused VectorE op.  Measured balance ~51/57/45us (ACT/DVE/Pool) per
    iteration; Pool ops stretch under DVE shared-SBUF-port contention, so
    pushing pass2 work to DVE (STT or a 2x custom op) measured WORSE.
  - x^T columns are host-permuted so psum partition p owns output rows
    s = 8p+j: a whole head's output is one [128, 8192] int8 staging tile
    stored with a single dma_start (dma_start costs ~0.6-4us of sequencer
    descriptor-generation time; 60 small DMAs saturated the SP ring, and
    alternating rings (DMA_ALT) stalls ScalarE compute - keep all on SP)
Loop-slope A/B (hardware, same-process interleaved, min-of-reps):
    v1 two-pass split-engine baseline:    96-102 us/iter (graded 548503 ns)
    + 4-quad psum, Pool pass2, fused op:  60-76 us/iter
    + h2-interleaved matmul emission:     ~1.06x further (final 64.5 us/iter
      with exact output in the same test.py run; absolute scale drifts
      ~25% between runs - within-run ratios are the trusted signal)
"""

from contextlib import ExitStack
import numpy as np
import ml_dtypes

import concourse.bacc as bacc
import concourse.tile as tile
from concourse import mybir
from concourse import dve_ops as _dve_ops
from concourse.bass_utils import run_bass_kernel_spmd
from concourse.dve_spec import C0, C1, C2, Spec, Src0, Zero, lower
from concourse.dve_uop import DveOpSpec

AF = mybir.ActivationFunctionType
OP = mybir.AluOpType
BF16 = mybir.dt.bfloat16


def _trunc_requant_ref(in0, in1, s0, s1, imm2):
    """Faithful fp32 emulation of TRUNC_REQUANT_ANT for CoreSim."""
    f32 = np.float32
    u = (in0.astype(f32) * f32(s0)).astype(f32) + f32(s1)
    u = u.astype(f32)
    ind = (u < f32(-imm2)).astype(f32) - (u > f32(imm2)).astype(f32)
    return (u + (f32(imm2) * ind).astype(f32)).astype(f32)


def _register_trunc_op():
    """Custom DVE uop: single-pass exact requant+trunc, psum fp32 -> int8.

        u   = fl(fl(q * s2) + zp)               (s2 = s * (1 + 2^-22))
        out = sat_i8(RNE(u + 0.5*((u < -0.5) - (u > 0.5))))

    The three-zone correction shifts u by +-0.5 so RNE lands on
    trunc-toward-zero; the middle zone gives the double-width output bin at
    0 that no single affine+RNE can produce.  Validated exhaustively on the
    host over every reachable q in [-1106688, 1047552]: 0 mismatches vs the
    reference fp32 chain (including saturation; RNE half-to-even ties fixed
    by the 2^-22 scale inflation, symmetric for trunc).
    """
    name = "TRUNC_REQUANT_ANT"
    for op in _dve_ops.OPS:
        if op.name == name:
            return op
    _u = Src0 * C0 + C1
    spec = Spec(
        body=_u + C2 * ((_u < -C2) - (_u > C2)),
        reference=_trunc_requant_ref,
    )
    row = max(_dve_ops._SUB_OPCODE_FOR_NAME.values()) + 1
    assert row < 0x20
    shas = {}
    for ver in ("v3", "v4"):
        try:
            uops = lower(spec, ver=ver)
            shas[ver] = DveOpSpec(name=name, opcode=row, uops=uops,
                                  rd1_en=False).sha(ver)
        except Exception:
            pass
    op = _dve_ops.DveOp(name=name, spec=spec, subdim=False, uops_sha=shas)
    _dve_ops.OPS.append(op)
    _dve_ops.CUSTOM_DVE_SPECS[name] = spec
    _dve_ops._SUB_OPCODE_FOR_NAME[name] = row
    return op


TRUNC_OP = _register_trunc_op()


def _pass2_ref(in0, in1, s0, s1, imm2):
    f32 = np.float32
    a = in0.astype(f32)
    return (a + (a < f32(0.0)).astype(f32)).astype(f32)


def _register_pass2_op():
    """Custom DVE uop for pass2: out_i8 = sat(RNE(A + [A<0])), A int16.
    Single-src, 16-bit -> eligible for the 2x_1P perf slot (perf_en), which
    doesn't touch the DVE/Pool shared SBUF port pair."""
    name = "P2_TRUNC_ANT"
    for op in _dve_ops.OPS:
        if op.name == name:
            return op
    spec = Spec(body=Src0 + (Src0 < Zero), reference=_pass2_ref)
    row = max(_dve_ops._SUB_OPCODE_FOR_NAME.values()) + 1
    assert row < 0x20
    shas = {}
    perf = {}
    for ver in ("v3", "v4"):
        try:
            uops = lower(spec, ver=ver)
            shas[ver] = DveOpSpec(name=name, opcode=row, uops=uops,
                                  rd1_en=False).sha(ver)
            perf[ver] = True
        except Exception:
            pass
    op = _dve_ops.DveOp(name=name, spec=spec, subdim=False, uops_sha=shas,
                        perf_en=perf)
    _dve_ops.OPS.append(op)
    _dve_ops.CUSTOM_DVE_SPECS[name] = spec
    _dve_ops._SUB_OPCODE_FOR_NAME[name] = row
    return op


P2_OP = _register_pass2_op()

N_CORES = 8
B, H, S, D = 8, 12, 1024, 64
HEADS_PER_CORE = B * H // N_CORES          # 12
N_PAIRS = HEADS_PER_CORE // 2              # 6
M_BLOCKS = S // 128                        # 8
T = 1024

# set by kernel() for test.py / bench.py to inspect
LAST_RESULTS = None
LAST_PREP = None

_NC_CACHE = {}

# --- engine-assignment knobs (all values hardware-A/B'd; see docstring)
ILV_NH_IN = True        # interleave at (h2,nh)-pair granularity: each lhsT
                        # serves its two N=512 matmuls consecutively (halves
                        # LDWEIGHTS reloads; row-group concurrency preserved
                        # at pair level)
H2_ILV = True           # interleave h0/h1 matmul emission for PE row-group
                        # concurrency (PE 41us -> ~21us busy; also removes
                        # any HAM rethrottle risk from PE idle gaps)
P2_DVE_OF = 0           # ACT-group pass2 ops moved to the DVE 2x custom op
                        # (measured monotonically WORSE: 4->+21us, 25->+46us
                        # per iter - DVE is the co-busiest engine)
SPLIT_JI = False        # per-group quad split ji0->ACT+Pool / ji1->fused
                        # DVE (measured +19us/iter worse: every group having
                        # adjacent DVE+Pool ops maximizes the shared-port
                        # contention stretch on Pool)
ACT_G_OF_48 = 25        # of the 48 groups, how many take the ScalarE
                        # two-pass path (ACT pass1 int16 + Pool pass2);
                        # the rest use the fused single-pass custom DVE op
                        # (TRUNC_REQUANT_ANT, psum fp32 -> int8 directly)
A_BUFS = 6              # [128, 2048] int16 pass1 staging tiles
ACT_BY_H2 = False       # group->engine by head instead of Bresenham:
                        # h0 -> ACT two-pass, h1 -> fused DVE (24/24) so the
                        # two concurrent drains of every j-block are always
                        # on different engines
P2_ACT_OF = 0           # of the ACT-groups' pass2 ops, how many run on
                        # ScalarE (activation Copy(A*c_b + d_b), identical
                        # numerics) instead of Pool - trades stretched Pool
                        # time (~2.9us/op under DVE port contention) for ACT
                        # slack (~2.0us/op)
P2_HALVES = False       # Pool pass2 as two [128,1024] ops per group (can
                        # start after the first ACT quad-drain; finer Pool
                        # queue granularity) vs one [128,2048] op
OB_BUFS = 4             # [128, 8192] int8 staging tiles (one per head;
                        # 4th buffer relieves a store-WAR stall under the
                        # interleaved emission - measured -4.7us/iter)
XIN_BUFS = 2            # packed [128, 2048] bf16 input tiles
DMA_ALT = False         # alternate dma_start between the two HWDGE rings
                        # (SP qSPDynamicHW / ACT qActDynamicHW) - descriptor
                        # generation is ~1.7-4us of sequencer time per DMA
                        # and saturates a single ring
DEFER_N = 2             # how many groups the pass2 trails its pass1
NH_SPLIT = 2            # rhs blocks per ji (N=512 moving operand)
SPLIT_FIRST_LOAD = True

# bench-only ablation knobs (correctness-invalid when set; used by ab.py to
# attribute loop-slope time to pipeline stages)
BENCH_NO_STORE = False
BENCH_NO_LOAD = False
BENCH_NO_MM = False
BENCH_NO_P2 = False
BENCH_NO_P1 = False


def _build_core_program(s_const: float, bias_a: float, c_b: float, d_b: float,
                        loop_iters: int | None = None,
                        bench_io: bool = False):
    """One NeuronCore's program: 12 heads of [1024,64]@[64,1024] + requant.

    loop_iters: when set, wraps the whole body in a hardware For_i loop -
    used only for benchmarking (device time scales with the loop count so a
    slope isolates HW exec time from host/relay dispatch overhead).

    bench_io: all big DRAM tensors become Internal (device-resident garbage,
    never shipped over the axon relay) and a tiny [1,16] ExternalOutput is
    added so PJRT has something to return.  Timing is data-independent, so
    the loop slope is unaffected; per-call payload drops from ~15 MB/core to
    16 bytes.
    """
    nc = bacc.Bacc("TRN2", target_bir_lowering=False, debug=False)
    io_kind_in = "Internal" if bench_io else "ExternalInput"
    io_kind_out = "Internal" if bench_io else "ExternalOutput"
    # head-pairs stacked on the partition axis; xt and yp packed into one
    # tensor so each pair needs a single input dma_start (SP sequencer DGE
    # config is ~1.7us per dma_start - the dominant serial cost at 60 DMAs)
    d_xy = nc.dram_tensor("xy", [N_PAIRS, 128, S + T], BF16, kind=io_kind_in)
    d_o = nc.dram_tensor("o", [HEADS_PER_CORE, S, T], mybir.dt.int8,
                         kind=io_kind_out)
    d_tick = (nc.dram_tensor("tick", [1, 16], mybir.dt.int8,
                             kind="ExternalOutput") if bench_io else None)

    with tile.TileContext(nc) as tc:
        with ExitStack() as stk:
            if loop_iters is not None:
                # PE's body exceeds one IRAM block; hint the back-edge so the
                # benchmark loop doesn't pay a ~3-4 us ifetch per iteration
                # that single-shot execution would not pay.
                stk.enter_context(tc.For_i(0, loop_iters, 1,
                                           hint_engines=(mybir.EngineType.PE,)))
            _emit_body(nc, tc, d_xy, d_o, s_const, bias_a, c_b, d_b)
        if d_tick is not None:
            with tc.tile_pool(name="tick", bufs=1) as tkpool:
                tk = tkpool.tile([1, 16], mybir.dt.int8, tag="tick")
                nc.vector.memset(tk[:], 0)
                nc.sync.dma_start(d_tick[:, :], tk[:])
    nc.compile()
    return nc


def _emit_body(nc, tc, d_xy, d_o, s_const, bias_a, c_b, d_b):
    # VectorE one-op pass1 constants (validated in validate_requant.py over
    # every reachable q: 0 mismatches, max tie margin variant)
    s64 = np.float64(np.float32(s_const))
    zp64 = np.float64(np.float32(bias_a)) + 0.5 - 2.0 ** -18  # recover out_zp
    b0 = float(np.float32((zp64 - 0.5) / s64 + 2.0 ** -8))
    s2 = float(np.float32(s64 * (1.0 + 2.0 ** -22)))

    # fused-op constants (exhaustively validated, see _register_trunc_op)
    s2c = float(np.float32(np.float64(np.float32(s_const)) *
                           (1.0 + 2.0 ** -22)))
    b2c = float(np.float32(np.float64(np.float32(bias_a)) + 0.5 - 2.0 ** -18))

    gidx = 0            # group counter (0..47)
    act_g = 0
    pending = []
    dma_i = [0]

    def dma(dst, src):
        # alternate between the two HWDGE rings; Tile still tracks deps
        eng = nc.sync if (not DMA_ALT or dma_i[0] % 2 == 0) else nc.scalar
        dma_i[0] += 1
        eng.dma_start(dst, src)
    with tc.tile_pool(name="xin", bufs=XIN_BUFS) as xpool, \
         tc.tile_pool(name="aint", bufs=A_BUFS) as apool, \
         tc.tile_pool(name="obuf", bufs=OB_BUFS) as opool, \
         tc.tile_pool(name="ps", bufs=4, space="PSUM") as pspool:
        tiles = [None] * N_PAIRS
        xy0 = xpool.tile([128, S + T], BF16, tag="xy")
        if BENCH_NO_LOAD:
            # bench-only: no HBM input traffic; one memset tile feeds every
            # head-pair (DVE ~0.9us/iter pollution, noted in readings)
            nc.vector.memset(xy0[:], 0)
        elif SPLIT_FIRST_LOAD:
            # single-shot prologue: land group-0's operands first so the
            # first matmuls start earlier (xt cols 0:256 + all of yp)
            dma(xy0[:, 0:256], d_xy[0, :, 0:256])
            dma(xy0[:, S:S + T], d_xy[0, :, S:S + T])
            dma(xy0[:, 256:S], d_xy[0, :, 256:S])
        else:
            dma(xy0[:], d_xy[0, :, :])
        tiles[0] = xy0

        ps_dummy = None
        if BENCH_NO_MM:
            # bench-only: pass1 reads this one pre-written quad (keeps the
            # psum-read cost while removing the matmul fills)
            ps_dummy = pspool.tile([128, 1024], mybir.dt.float32, tag="ps")
            nc.vector.memset(ps_dummy[:], 0)

        for pair in range(N_PAIRS):
            xy_t = tiles[pair]
            xt_t = xy_t[:, 0:S]
            yp_t = xy_t[:, S:S + T]
            if pair + 1 < N_PAIRS:
                # prefetch next pair's operands now so the loads sit ahead
                # of this pair's output stores in the SP HWDGE FIFO
                if BENCH_NO_LOAD:
                    tiles[pair + 1] = xy0
                else:
                    xy_n = xpool.tile([128, S + T], BF16, tag="xy")
                    dma(xy_n[:], d_xy[pair + 1, :, :])
                    tiles[pair + 1] = xy_n

            ob = {}

            for jg in range(M_BLOCKS // 2):
                if H2_ILV:
                    # interleave the two heads' matmuls so adjacent PE
                    # instructions target different row-groups (0,0)/(64,0)
                    # and execute concurrently (~2x effective PE rate);
                    # drains per quad follow right after its two matmuls.
                    gs = []
                    for h2 in range(2):
                        if ACT_BY_H2:
                            # strict per-j-block engine pairing (h0->ACT,
                            # h1->DVE) keeping the 25/23 ratio: one extra
                            # ACT group mid-iteration (group 25 -> h1 slot)
                            want_act = (h2 == 0) or (gidx == 25)
                            gidx += 1
                            if want_act:
                                act_g += 1
                        else:
                            want_act = ((gidx + 1) * ACT_G_OF_48) // 48 \
                                > act_g
                            gidx += 1
                            if want_act:
                                act_g += 1
                        a_t = (apool.tile([128, 2048], mybir.dt.int16,
                                          tag="a",
                                          name=f"a_{pair}_{h2}_{jg}")
                               if want_act else None)
                        if h2 not in ob:
                            ob[h2] = opool.tile([128, 8192], mybir.dt.int8,
                                                tag="obs",
                                                name=f"ob_{pair}_{h2}")
                        ob_t = ob[h2][:, jg * 2048:(jg + 1) * 2048]
                        gs.append((h2, want_act, a_t, ob_t))
                    for ji in range(2):
                        j = jg * 2 + ji
                        psjs = {}
                        for h2, _, _, _ in gs:
                            psjs[h2] = pspool.tile(
                                [128, 1024], mybir.dt.float32, tag="ps",
                                name=f"ps_{pair}_{h2}_{jg}_{ji}")
                        nw = 1024 // NH_SPLIT
                        mm_order = ([(h2, nh) for h2 in range(2)
                                     for nh in range(NH_SPLIT)]
                                    if ILV_NH_IN else
                                    [(h2, nh) for nh in range(NH_SPLIT)
                                     for h2 in range(2)])
                        for h2, nh in mm_order:
                            nc.tensor.matmul(
                                psjs[h2][:, nh * nw:(nh + 1) * nw],
                                xt_t[64 * h2:64 * h2 + 64,
                                     j * 128:(j + 1) * 128],
                                yp_t[64 * h2:64 * h2 + 64,
                                     nh * nw:(nh + 1) * nw],
                                start=True, stop=True,
                                tile_position=(64 * h2, 0),
                            )
                        for h2, want_act, a_t, ob_t in gs:
                            if want_act:
                                nc.scalar.activation(
                                    a_t[:, ji * 1024:(ji + 1) * 1024],
                                    psjs[h2][:],
                                    AF.Copy, bias=bias_a, scale=s_const)
                                if P2_HALVES:
                                    lo, hi = ji * 1024, (ji + 1) * 1024

                                    def fmap(a_t=a_t, ob_t=ob_t,
                                             lo=lo, hi=hi):
                                        nc.gpsimd.tensor_scalar(
                                            ob_t[:, lo:hi], a_t[:, lo:hi],
                                            c_b, d_b, OP.mult, OP.add)
                                    pending.append(fmap)
                            else:
                                nc.vector._custom_dve(
                                    TRUNC_OP,
                                    out=ob_t[:, ji * 1024:(ji + 1) * 1024],
                                    in0=psjs[h2][:], s0=s2c, s1=b2c,
                                    imm2=0.5)
                    for h2, want_act, a_t, ob_t in gs:
                        if want_act and not P2_HALVES:
                            if (act_g - 1) % 25 < P2_ACT_OF:
                                def fmap(a_t=a_t, ob_t=ob_t):
                                    nc.scalar.activation(
                                        ob_t[:], a_t[:], AF.Copy,
                                        bias=d_b, scale=c_b)
                            else:
                                def fmap(a_t=a_t, ob_t=ob_t):
                                    nc.gpsimd.tensor_scalar(
                                        ob_t[:], a_t[:], c_b, d_b,
                                        OP.mult, OP.add)
                            pending.append(fmap)
                    while len(pending) > DEFER_N:
                        pending.pop(0)()
                    continue
                for h2 in range(2):
                    # per-group engine split: ACT two-pass (+Pool pass2) vs
                    # fused single-pass custom DVE op straight to int8
                    want_act = ((gidx + 1) * ACT_G_OF_48) // 48 > act_g
                    gidx += 1
                    if want_act:
                        act_g += 1
                    need_a = (SPLIT_JI or want_act) and not BENCH_NO_P1
                    a_cols = 1024 if SPLIT_JI else 2048
                    a_t = (apool.tile([128, a_cols], mybir.dt.int16,
                                      tag="a", name=f"a_{pair}_{h2}_{jg}")
                           if need_a else None)
                    if h2 not in ob:
                        ob[h2] = opool.tile([128, 8192], mybir.dt.int8,
                                            tag="obs",
                                            name=f"ob_{pair}_{h2}")
                        if BENCH_NO_P1 or BENCH_NO_P2:
                            # bench-only: ensure the store has a writer even
                            # when the producing ops are ablated
                            nc.vector.memset(ob[h2][:, 0:8192], 0)
                    ob_t = ob[h2][:, jg * 2048:(jg + 1) * 2048]
                    for ji in range(2):
                        j = jg * 2 + ji
                        lhsT = xt_t[64 * h2:64 * h2 + 64,
                                    j * 128:(j + 1) * 128]
                        psj = (ps_dummy if BENCH_NO_MM else
                               pspool.tile([128, 1024], mybir.dt.float32,
                                           tag="ps",
                                           name=f"ps_{pair}_{h2}_{jg}_{ji}"))
                        for nh in (() if BENCH_NO_MM else range(NH_SPLIT)):
                            nw = 1024 // NH_SPLIT
                            nc.tensor.matmul(
                                psj[:, nh * nw:(nh + 1) * nw],
                                lhsT,
                                yp_t[64 * h2:64 * h2 + 64,
                                     nh * nw:(nh + 1) * nw],
                                start=True, stop=True,
                                tile_position=(64 * h2, 0),
                            )
                        # drain this quad now; 4-buffer rotation keeps the
                        # fills decoupled
                        quad_act = (ji == 0) if SPLIT_JI else want_act
                        if BENCH_NO_P1:
                            pass
                        elif quad_act:
                            a_dst = (a_t[:, 0:1024] if SPLIT_JI else
                                     a_t[:, ji * 1024:(ji + 1) * 1024])
                            nc.scalar.activation(
                                a_dst, psj[:],
                                AF.Copy, bias=bias_a, scale=s_const)
                        else:
                            nc.vector._custom_dve(
                                TRUNC_OP,
                                out=ob_t[:, ji * 1024:(ji + 1) * 1024],
                                in0=psj[:], s0=s2c, s1=b2c, imm2=0.5)
                    # Pool pass2 for the ACT-drained columns, deferred so
                    # it trails pass1 by DEFER_N groups
                    if (SPLIT_JI or want_act) and not BENCH_NO_P1 \
                            and not BENCH_NO_P2:
                        p2_dst = ob_t[:, 0:1024] if SPLIT_JI else ob_t[:]
                        on_dve = (act_g - 1) % 25 < P2_DVE_OF \
                            if want_act else False
                        if on_dve:
                            def fmap(a_t=a_t, p2_dst=p2_dst):
                                nc.vector._custom_dve(
                                    P2_OP, out=p2_dst, in0=a_t[:])
                        else:
                            def fmap(a_t=a_t, p2_dst=p2_dst):
                                nc.gpsimd.tensor_scalar(p2_dst, a_t[:],
                                                        c_b, d_b,
                                                        OP.mult, OP.add)
                        pending.append(fmap)
                    while len(pending) > DEFER_N:
                        pending.pop(0)()
            # flush this pair's remaining pass2 ops, then batched output DMAs
            while pending:
                pending.pop(0)()
            for h2 in range(2):
                if BENCH_NO_STORE:
                    continue
                dst = d_o[2 * pair + h2, :, :].rearrange(
                    "(p j) t -> p (j t)", j=M_BLOCKS)
                dma(dst[:, :], ob[h2][:, 0:8192])


def default_key():
    """Requant constants for the reference problem's quantization params -
    used by bench.py when kernel() hasn't run in this process."""
    s_const = float(np.float32(np.float32(0.000234) / np.float32(0.0625)))
    bias_a = float(np.float64(np.float32(2.0)) - 0.5 + 2.0 ** -18)
    c_b = float(np.float32(255.0 / 256.0))
    d_b = float(np.float32(0.499))
    return (s_const, bias_a, c_b, d_b)


def kernel(x, y, alpha, a_zp, b_zp, out_zp, o_alpha):
    global LAST_RESULTS, LAST_PREP
    x = np.asarray(x)
    y = np.asarray(y)
    s_const = float(np.float32(np.float32(alpha) / np.float32(o_alpha)))
    bias_a = float(np.float64(np.float32(out_zp)) - 0.5 + 2.0 ** -18)
    c_b = float(np.float32(255.0 / 256.0))
    d_b = float(np.float32(0.499))

    # ---- host-side shard + dequant prep (exact in bf16) ----
    xf = x.reshape(B * H, S, D).astype(np.float32) - np.float32(a_zp)
    yf = y.reshape(B * H, D, T).astype(np.float32) - np.float32(b_zp)
    # lhsT layout: [head, D, S], head-pairs stacked to 128 partitions.
    # S-columns permuted to c = j*128 + p  <->  s = 8p + j so each psum
    # partition owns 8 consecutive output rows (8 KiB DMA runs).
    xt = np.ascontiguousarray(xf.transpose(0, 2, 1)).astype(ml_dtypes.bfloat16)
    xt = np.ascontiguousarray(
        xt.reshape(B * H, D, S // 8, 8).transpose(0, 1, 3, 2)).reshape(
        B * H, D, S)
    yp = yf.astype(ml_dtypes.bfloat16)
    xt = xt.reshape(N_CORES, N_PAIRS, 128, S)
    yp = yp.reshape(N_CORES, N_PAIRS, 128, T)
    # pack [xt | yp] so each pair is one input dma_start on-device
    xy = np.concatenate([xt, yp], axis=-1)

    key = (s_const, bias_a, c_b, d_b)
    if key not in _NC_CACHE:
        _NC_CACHE[key] = _build_core_program(*key)
    nc = _NC_CACHE[key]

    in_maps = [{"xy": xy[c]} for c in range(N_CORES)]
    LAST_PREP = (key, in_maps)
    res = run_bass_kernel_spmd(nc, in_maps, core_ids=list(range(N_CORES)))
    LAST_RESULTS = res

    out = np.stack([res.results[c]["o"] for c in range(N_CORES)])
    return out.reshape(B, H, S, T)


if __name__ == "__main__":
    rng = np.random.default_rng(0)
    x = rng.integers(-128, 128, size=(B, H, S, D)).astype(np.int32)
    y = rng.integers(-128, 128, size=(B, H, D, T)).astype(np.int32)
    out = kernel(x=x, y=y, alpha=np.float32(0.000234), a_zp=np.float32(3.0),
                 b_zp=np.float32(-5.0), out_zp=np.float32(2.0),
                 o_alpha=np.float32(0.0625))
    print("kernel output", out.shape, out.dtype)



# revision 48
# speedup vs baseline: 1.2765x; 1.0532x over previous
"""Trainium2 Bass kernel for nn_BMMS8TS8NS8T: batched int8-valued GEMM with
dequant/requant, sharded head-parallel across 8 NeuronCores.

Reference semantics (jax CPU, fp32):
    a = x.float() - a_zp          # [B,H,S,D]  int8-valued
    b = y.float() - b_zp          # [B,H,D,T]
    q = a @ b                     # exact integers (|q| <= 64*131*132 < 2^24)
    v = fl(fl(q * s) + out_zp),   s = fl(alpha / o_alpha)
    out = trunc(clip(v, -128, 127)).astype(int8)   # trunc toward zero

Device strategy per core (12 heads = (B*H)/8, head parallel, no cross-core
communication):
  - host pre-dequantizes inputs to bf16 (exact: integers with |v| <= 132 are
    exactly representable) and pre-transposes/permutes x so the stationary
    matmul operand needs no on-device transpose; x^T and y are packed into
    ONE dram tensor per head-pair so each pair costs one input dma_start
  - TensorE: K=64 matmuls, two heads packed in the 128-row PE array via row
    tiling (tile_position (0,0)/(64,0)); fp32 PSUM accumulation is exact.
    The two heads' matmuls are emitted INTERLEAVED (H2_ILV) so adjacent PE
    instructions land on different row-groups and execute concurrently
    (~2x effective PE rate, earlier drain starts; measured -4us/iter)
  - PSUM as 4 rotating [128,1024] quads (bufs=4) so matmul fills decouple
    from drains (2x[128,2048] ping-pong was period-bound: a tile's refill
    waited on its own ~2.1us drain -> ~71us/iter cap)
  - requantization: the key insight is a CUSTOM DVE uop (per-NEFF uop table,
    registered at import into concourse.dve_ops) that does the whole
    requant+trunc in ONE 1x pass from PSUM fp32 straight to int8:
        u   = fl(fl(q * s2) + zp),  s2 = fl(s * (1 + 2^-22))
        out = sat_i8(RNE(u + 0.5*((u < -0.5) - (u > 0.5))))
    The three-zone +-0.5 shift makes RNE produce trunc-toward-zero incl.
    the double-width bin at 0 that no single affine+RNE can express; the
    2^-22 scale inflation breaks RNE half-to-even ties symmetrically
    (correct for trunc on both signs).  Validated exhaustively on host over
    every reachable q: 0 mismatches (saturation included); hardware runs
    bit-exact vs the jax reference.
  - 1x PSUM-read capacity is the structural floor (GpSimd has NO PSUM port;
    fp32 PSUM reads are 1x on ScalarE/VectorE -> >= 12.6M reads /
    276G elem/s ~= 46us/iter minimum).  To use all three post-PE engines,
    ACT_G_OF_48 groups take a two-pass path (ScalarE pass1 -> int16,
    GpSimd/Pool pass2 tensor_scalar A*(255/256)+0.499) and the rest use the
    fused VectorE op.  Measured balance ~51/57/45us (ACT/DVE/Pool) per
    iteration; Pool ops stretch under DVE shared-SBUF-port contention, so
    pushing pass2 work to DVE (STT or a 2x custom op) measured WORSE.
  - x^T columns are host-permuted so psum partition p owns output rows
    s = 8p+j: a whole head's output is one [128, 8192] int8 staging tile
    stored with a single dma_start (dma_start costs ~0.6-4us of sequencer
    descriptor-generation time; 60 small DMAs saturated the SP ring, and
    alternating rings (DMA_ALT) stalls ScalarE compute - keep all on SP)
Loop-slope A/B (hardware, same-process interleaved, min-of-reps):
    v1 two-pass split-engine baseline:    96-102 us/iter (graded 548503 ns)
    + 4-quad psum, Pool pass2, fused op:  60-76 us/iter
    + h2-interleaved matmul emission:     ~1.06x further (final 64.5 us/iter
      with exact output in the same test.py run; absolute scale drifts
      ~25% between runs - within-run ratios are the trusted signal)
"""

from contextlib import ExitStack
import numpy as np
import ml_dtypes

import concourse.bacc as bacc
import concourse.tile as tile
from concourse import mybir
from concourse import dve_ops as _dve_ops
from concourse.bass_utils import run_bass_kernel_spmd
from concourse.dve_spec import C0, C1, C2, Spec, Src0, Zero, lower
from concourse.dve_uop import DveOpSpec

AF = mybir.ActivationFunctionType
OP = mybir.AluOpType
BF16 = mybir.dt.bfloat16


def _trunc_requant_ref(in0, in1, s0, s1, imm2):
    """Faithful fp32 emulation of TRUNC_REQUANT_ANT for CoreSim."""
    f32 = np.float32
    u = (in0.astype(f32) * f32(s0)).astype(f32) + f32(s1)
    u = u.astype(f32)
    ind = (u < f32(-imm2)).astype(f32) - (u > f32(imm2)).astype(f32)
    return (u + (f32(imm2) * ind).astype(f32)).astype(f32)


def _register_trunc_op():
    """Custom DVE uop: single-pass exact requant+trunc, psum fp32 -> int8.

        u   = fl(fl(q * s2) + zp)               (s2 = s * (1 + 2^-22))
        out = sat_i8(RNE(u + 0.5*((u < -0.5) - (u > 0.5))))

    The three-zone correction shifts u by +-0.5 so RNE lands on
    trunc-toward-zero; the middle zone gives the double-width output bin at
    0 that no single affine+RNE can produce.  Validated exhaustively on the
    host over every reachable q in [-1106688, 1047552]: 0 mismatches vs the
    reference fp32 chain (including saturation; RNE half-to-even ties fixed
    by the 2^-22 scale inflation, symmetric for trunc).
    """
    name = "TRUNC_REQUANT_ANT"
    for op in _dve_ops.OPS:
        if op.name == name:
            return op
    _u = Src0 * C0 + C1
    spec = Spec(
        body=_u + C2 * ((_u < -C2) - (_u > C2)),
        reference=_trunc_requant_ref,
    )
    row = max(_dve_ops._SUB_OPCODE_FOR_NAME.values()) + 1
    assert row < 0x20
    shas = {}
    for ver in ("v3", "v4"):
        try:
            uops = lower(spec, ver=ver)
            shas[ver] = DveOpSpec(name=name, opcode=row, uops=uops,
                                  rd1_en=False).sha(ver)
        except Exception:
            pass
    op = _dve_ops.DveOp(name=name, spec=spec, subdim=False, uops_sha=shas)
    _dve_ops.OPS.append(op)
    _dve_ops.CUSTOM_DVE_SPECS[name] = spec
    _dve_ops._SUB_OPCODE_FOR_NAME[name] = row
    return op


TRUNC_OP = _register_trunc_op()


def _pass2_ref(in0, in1, s0, s1, imm2):
    f32 = np.float32
    a = in0.astype(f32)
    return (a + (a < f32(0.0)).astype(f32)).astype(f32)


def _register_pass2_op():
    """Custom DVE uop for pass2: out_i8 = sat(RNE(A + [A<0])), A int16.
    Single-src, 16-bit -> eligible for the 2x_1P perf slot (perf_en), which
    doesn't touch the DVE/Pool shared SBUF port pair."""
    name = "P2_TRUNC_ANT"
    for op in _dve_ops.OPS:
        if op.name == name:
            return op
    spec = Spec(body=Src0 + (Src0 < Zero), reference=_pass2_ref)
    row = max(_dve_ops._SUB_OPCODE_FOR_NAME.values()) + 1
    assert row < 0x20
    shas = {}
    perf = {}
    for ver in ("v3", "v4"):
        try:
            uops = lower(spec, ver=ver)
            shas[ver] = DveOpSpec(name=name, opcode=row, uops=uops,
                                  rd1_en=False).sha(ver)
            perf[ver] = True
        except Exception:
            pass
    op = _dve_ops.DveOp(name=name, spec=spec, subdim=False, uops_sha=shas,
                        perf_en=perf)
    _dve_ops.OPS.append(op)
    _dve_ops.CUSTOM_DVE_SPECS[name] = spec
    _dve_ops._SUB_OPCODE_FOR_NAME[name] = row
    return op


P2_OP = _register_pass2_op()

N_CORES = 8
B, H, S, D = 8, 12, 1024, 64
HEADS_PER_CORE = B * H // N_CORES          # 12
N_PAIRS = HEADS_PER_CORE // 2              # 6
M_BLOCKS = S // 128                        # 8
T = 1024

# set by kernel() for test.py / bench.py to inspect
LAST_RESULTS = None
LAST_PREP = None

_NC_CACHE = {}

# --- engine-assignment knobs (all values hardware-A/B'd; see docstring)
ILV_NH_IN = True        # interleave at (h2,nh)-pair granularity: each lhsT
                        # serves its two N=512 matmuls consecutively (halves
                        # LDWEIGHTS reloads; row-group concurrency preserved
                        # at pair level)
H2_ILV = True           # interleave h0/h1 matmul emission for PE row-group
                        # concurrency (PE 41us -> ~21us busy; also removes
                        # any HAM rethrottle risk from PE idle gaps)
P2_DVE_OF = 0           # ACT-group pass2 ops moved to the DVE 2x custom op
                        # (measured monotonically WORSE: 4->+21us, 25->+46us
                        # per iter - DVE is the co-busiest engine)
SPLIT_JI = False        # per-group quad split ji0->ACT+Pool / ji1->fused
                        # DVE (measured +19us/iter worse: every group having
                        # adjacent DVE+Pool ops maximizes the shared-port
                        # contention stretch on Pool)
ACT_G_OF_48 = 25        # of the 48 groups, how many take the ScalarE
                        # two-pass path (ACT pass1 int16 + Pool pass2);
                        # the rest use the fused single-pass custom DVE op
                        # (TRUNC_REQUANT_ANT, psum fp32 -> int8 directly)
A_BUFS = 6              # [128, 2048] int16 pass1 staging tiles
ACT_BY_H2 = False       # group->engine by head instead of Bresenham:
                        # h0 -> ACT two-pass, h1 -> fused DVE (24/24) so the
                        # two concurrent drains of every j-block are always
                        # on different engines
P2_ACT_OF = 0           # of the ACT-groups' pass2 ops, how many run on
                        # ScalarE (activation Copy(A*c_b + d_b), identical
                        # numerics) instead of Pool - trades stretched Pool
                        # time (~2.9us/op under DVE port contention) for ACT
                        # slack (~2.0us/op)
P2_HALVES = False       # Pool pass2 as two [128,1024] ops per group (can
                        # start after the first ACT quad-drain; finer Pool
                        # queue granularity) vs one [128,2048] op
OB_BUFS = 4             # [128, 8192] int8 staging tiles (one per head;
                        # 4th buffer relieves a store-WAR stall under the
                        # interleaved emission - measured -4.7us/iter)
XIN_BUFS = 2            # packed [128, 2048] bf16 input tiles
DMA_ALT = False         # alternate dma_start between the two HWDGE rings
                        # (SP qSPDynamicHW / ACT qActDynamicHW) - descriptor
                        # generation is ~1.7-4us of sequencer time per DMA
                        # and saturates a single ring
DEFER_N = 2             # how many groups the pass2 trails its pass1
NH_SPLIT = 2            # rhs blocks per ji (N=512 moving operand)
SPLIT_FIRST_LOAD = True

# bench-only ablation knobs (correctness-invalid when set; used by ab.py to
# attribute loop-slope time to pipeline stages)
BENCH_NO_STORE = False
BENCH_NO_LOAD = False
BENCH_NO_MM = False
BENCH_NO_P2 = False
BENCH_NO_P1 = False
LOOP_HINT_ALL = False   # hint the For_i back-edge on every engine (not just
                        # PE): if ACT/DVE/Pool bodies exceed one IRAM block,
                        # each loop iteration pays a multi-us ifetch without
                        # the hint


def _build_core_program(s_const: float, bias_a: float, c_b: float, d_b: float,
                        loop_iters: int | None = None,
                        bench_io: bool = False):
    """One NeuronCore's program: 12 heads of [1024,64]@[64,1024] + requant.

    loop_iters: when set, wraps the whole body in a hardware For_i loop -
    used only for benchmarking (device time scales with the loop count so a
    slope isolates HW exec time from host/relay dispatch overhead).

    bench_io: all big DRAM tensors become Internal (device-resident garbage,
    never shipped over the axon relay) and a tiny [1,16] ExternalOutput is
    added so PJRT has something to return.  Timing is data-independent, so
    the loop slope is unaffected; per-call payload drops from ~15 MB/core to
    16 bytes.
    """
    nc = bacc.Bacc("TRN2", target_bir_lowering=False, debug=False)
    io_kind_in = "Internal" if bench_io else "ExternalInput"
    io_kind_out = "Internal" if bench_io else "ExternalOutput"
    # head-pairs stacked on the partition axis; xt and yp packed into one
    # tensor so each pair needs a single input dma_start (SP sequencer DGE
    # config is ~1.7us per dma_start - the dominant serial cost at 60 DMAs)
    d_xy = nc.dram_tensor("xy", [N_PAIRS, 128, S + T], BF16, kind=io_kind_in)
    d_o = nc.dram_tensor("o", [HEADS_PER_CORE, S, T], mybir.dt.int8,
                         kind=io_kind_out)
    d_tick = (nc.dram_tensor("tick", [1, 16], mybir.dt.int8,
                             kind="ExternalOutput") if bench_io else None)

    with tile.TileContext(nc) as tc:
        with ExitStack() as stk:
            if loop_iters is not None:
                # PE's body exceeds one IRAM block; hint the back-edge so the
                # benchmark loop doesn't pay a ~3-4 us ifetch per iteration
                # that single-shot execution would not pay.
                hints = ((mybir.EngineType.PE, mybir.EngineType.Activation,
                          mybir.EngineType.DVE, mybir.EngineType.Pool,
                          mybir.EngineType.SP)
                         if LOOP_HINT_ALL else (mybir.EngineType.PE,))
                stk.enter_context(tc.For_i(0, loop_iters, 1,
                                           hint_engines=hints))
            _emit_body(nc, tc, d_xy, d_o, s_const, bias_a, c_b, d_b)
        if d_tick is not None:
            with tc.tile_pool(name="tick", bufs=1) as tkpool:
                tk = tkpool.tile([1, 16], mybir.dt.int8, tag="tick")
                nc.vector.memset(tk[:], 0)
                nc.sync.dma_start(d_tick[:, :], tk[:])
    nc.compile()
    return nc


def _emit_body(nc, tc, d_xy, d_o, s_const, bias_a, c_b, d_b):
    # VectorE one-op pass1 constants (validated in validate_requant.py over
    # every reachable q: 0 mismatches, max tie margin variant)
    s64 = np.float64(np.float32(s_const))
    zp64 = np.float64(np.float32(bias_a)) + 0.5 - 2.0 ** -18  # recover out_zp
    b0 = float(np.float32((zp64 - 0.5) / s64 + 2.0 ** -8))
    s2 = float(np.float32(s64 * (1.0 + 2.0 ** -22)))

    # fused-op constants (exhaustively validated, see _register_trunc_op)
    s2c = float(np.float32(np.float64(np.float32(s_const)) *
                           (1.0 + 2.0 ** -22)))
    b2c = float(np.float32(np.float64(np.float32(bias_a)) + 0.5 - 2.0 ** -18))

    gidx = 0            # group counter (0..47)
    act_g = 0
    pending = []
    dma_i = [0]

    def dma(dst, src):
        # alternate between the two HWDGE rings; Tile still tracks deps
        eng = nc.sync if (not DMA_ALT or dma_i[0] % 2 == 0) else nc.scalar
        dma_i[0] += 1
        eng.dma_start(dst, src)
    with tc.tile_pool(name="xin", bufs=XIN_BUFS) as xpool, \
         tc.tile_pool(name="aint", bufs=A_BUFS) as apool, \
         tc.tile_pool(name="obuf", bufs=OB_BUFS) as opool, \
         tc.tile_pool(name="ps", bufs=4, space="PSUM") as pspool:
        tiles = [None] * N_PAIRS
        xy0 = xpool.tile([128, S + T], BF16, tag="xy")
        if BENCH_NO_LOAD:
            # bench-only: no HBM input traffic; one memset tile feeds every
            # head-pair (DVE ~0.9us/iter pollution, noted in readings)
            nc.vector.memset(xy0[:], 0)
        elif SPLIT_FIRST_LOAD:
            # single-shot prologue: land group-0's operands first so the
            # first matmuls start earlier (xt cols 0:256 + all of yp)
            dma(xy0[:, 0:256], d_xy[0, :, 0:256])
            dma(xy0[:, S:S + T], d_xy[0, :, S:S + T])
            dma(xy0[:, 256:S], d_xy[0, :, 256:S])
        else:
            dma(xy0[:], d_xy[0, :, :])
        tiles[0] = xy0

        ps_dummy = None
        if BENCH_NO_MM:
            # bench-only: pass1 reads this one pre-written quad (keeps the
            # psum-read cost while removing the matmul fills)
            ps_dummy = pspool.tile([128, 1024], mybir.dt.float32, tag="ps")
            nc.vector.memset(ps_dummy[:], 0)

        for pair in range(N_PAIRS):
            xy_t = tiles[pair]
            xt_t = xy_t[:, 0:S]
            yp_t = xy_t[:, S:S + T]
            if pair + 1 < N_PAIRS:
                # prefetch next pair's operands now so the loads sit ahead
                # of this pair's output stores in the SP HWDGE FIFO
                if BENCH_NO_LOAD:
                    tiles[pair + 1] = xy0
                else:
                    xy_n = xpool.tile([128, S + T], BF16, tag="xy")
                    dma(xy_n[:], d_xy[pair + 1, :, :])
                    tiles[pair + 1] = xy_n

            ob = {}

            for jg in range(M_BLOCKS // 2):
                if H2_ILV:
                    # interleave the two heads' matmuls so adjacent PE
                    # instructions target different row-groups (0,0)/(64,0)
                    # and execute concurrently (~2x effective PE rate);
                    # drains per quad follow right after its two matmuls.
                    gs = []
                    for h2 in range(2):
                        if ACT_BY_H2:
                            # strict per-j-block engine pairing (h0->ACT,
                            # h1->DVE) keeping the 25/23 ratio: one extra
                            # ACT group mid-iteration (group 25 -> h1 slot)
                            want_act = (h2 == 0) or (gidx == 25)
                            gidx += 1
                            if want_act:
                                act_g += 1
                        else:
                            want_act = ((gidx + 1) * ACT_G_OF_48) // 48 \
                                > act_g
                            gidx += 1
                            if want_act:
                                act_g += 1
                        a_t = (apool.tile([128, 2048], mybir.dt.int16,
                                          tag="a",
                                          name=f"a_{pair}_{h2}_{jg}")
                               if want_act else None)
                        if h2 not in ob:
                            ob[h2] = opool.tile([128, 8192], mybir.dt.int8,
                                                tag="obs",
                                                name=f"ob_{pair}_{h2}")
                        ob_t = ob[h2][:, jg * 2048:(jg + 1) * 2048]
                        gs.append((h2, want_act, a_t, ob_t))
                    for ji in range(2):
                        j = jg * 2 + ji
                        psjs = {}
                        for h2, _, _, _ in gs:
                            psjs[h2] = pspool.tile(
                                [128, 1024], mybir.dt.float32, tag="ps",
                                name=f"ps_{pair}_{h2}_{jg}_{ji}")
                        nw = 1024 // NH_SPLIT
                        mm_order = ([(h2, nh) for h2 in range(2)
                                     for nh in range(NH_SPLIT)]
                                    if ILV_NH_IN else
                                    [(h2, nh) for nh in range(NH_SPLIT)
                                     for h2 in range(2)])
                        for h2, nh in mm_order:
                            nc.tensor.matmul(
                                psjs[h2][:, nh * nw:(nh + 1) * nw],
                                xt_t[64 * h2:64 * h2 + 64,
                                     j * 128:(j + 1) * 128],
                                yp_t[64 * h2:64 * h2 + 64,
                                     nh * nw:(nh + 1) * nw],
                                start=True, stop=True,
                                tile_position=(64 * h2, 0),
                            )
                        for h2, want_act, a_t, ob_t in gs:
                            if want_act:
                                nc.scalar.activation(
                                    a_t[:, ji * 1024:(ji + 1) * 1024],
                                    psjs[h2][:],
                                    AF.Copy, bias=bias_a, scale=s_const)
                                if P2_HALVES:
                                    lo, hi = ji * 1024, (ji + 1) * 1024

                                    def fmap(a_t=a_t, ob_t=ob_t,
                                             lo=lo, hi=hi):
                                        nc.gpsimd.tensor_scalar(
                                            ob_t[:, lo:hi], a_t[:, lo:hi],
                                            c_b, d_b, OP.mult, OP.add)
                                    pending.append(fmap)
                            else:
                                nc.vector._custom_dve(
                                    TRUNC_OP,
                                    out=ob_t[:, ji * 1024:(ji + 1) * 1024],
                                    in0=psjs[h2][:], s0=s2c, s1=b2c,
                                    imm2=0.5)
                    for h2, want_act, a_t, ob_t in gs:
                        if want_act and not P2_HALVES:
                            if (act_g - 1) % 25 < P2_ACT_OF:
                                def fmap(a_t=a_t, ob_t=ob_t):
                                    nc.scalar.activation(
                                        ob_t[:], a_t[:], AF.Copy,
                                        bias=d_b, scale=c_b)
                            else:
                                def fmap(a_t=a_t, ob_t=ob_t):
                                    nc.gpsimd.tensor_scalar(
                                        ob_t[:], a_t[:], c_b, d_b,
                                        OP.mult, OP.add)
                            pending.append(fmap)
                    while len(pending) > DEFER_N:
                        pending.pop(0)()
                    continue
                for h2 in range(2):
                    # per-group engine split: ACT two-pass (+Pool pass2) vs
                    # fused single-pass custom DVE op straight to int8
                    want_act = ((gidx + 1) * ACT_G_OF_48) // 48 > act_g
                    gidx += 1
                    if want_act:
                        act_g += 1
                    need_a = (SPLIT_JI or want_act) and not BENCH_NO_P1
                    a_cols = 1024 if SPLIT_JI else 2048
                    a_t = (apool.tile([128, a_cols], mybir.dt.int16,
                                      tag="a", name=f"a_{pair}_{h2}_{jg}")
                           if need_a else None)
                    if h2 not in ob:
                        ob[h2] = opool.tile([128, 8192], mybir.dt.int8,
                                            tag="obs",
                                            name=f"ob_{pair}_{h2}")
                        if BENCH_NO_P1 or BENCH_NO_P2:
                            # bench-only: ensure the store has a writer even
                            # when the producing ops are ablated
                            nc.vector.memset(ob[h2][:, 0:8192], 0)
                    ob_t = ob[h2][:, jg * 2048:(jg + 1) * 2048]
                    for ji in range(2):
                        j = jg * 2 + ji
                        lhsT = xt_t[64 * h2:64 * h2 + 64,
                                    j * 128:(j + 1) * 128]
                        psj = (ps_dummy if BENCH_NO_MM else
                               pspool.tile([128, 1024], mybir.dt.float32,
                                           tag="ps",
                                           name=f"ps_{pair}_{h2}_{jg}_{ji}"))
                        for nh in (() if BENCH_NO_MM else range(NH_SPLIT)):
                            nw = 1024 // NH_SPLIT
                            nc.tensor.matmul(
                                psj[:, nh * nw:(nh + 1) * nw],
                                lhsT,
                                yp_t[64 * h2:64 * h2 + 64,
                                     nh * nw:(nh + 1) * nw],
                                start=True, stop=True,
                                tile_position=(64 * h2, 0),
                            )
                        # drain this quad now; 4-buffer rotation keeps the
                        # fills decoupled
                        quad_act = (ji == 0) if SPLIT_JI else want_act
                        if BENCH_NO_P1:
                            pass
                        elif quad_act:
                            a_dst = (a_t[:, 0:1024] if SPLIT_JI else
                                     a_t[:, ji * 1024:(ji + 1) * 1024])
                            nc.scalar.activation(
                                a_dst, psj[:],
                                AF.Copy, bias=bias_a, scale=s_const)
                        else:
                            nc.vector._custom_dve(
                                TRUNC_OP,
                                out=ob_t[:, ji * 1024:(ji + 1) * 1024],
                                in0=psj[:], s0=s2c, s1=b2c, imm2=0.5)
                    # Pool pass2 for the ACT-drained columns, deferred so
                    # it trails pass1 by DEFER_N groups
                    if (SPLIT_JI or want_act) and not BENCH_NO_P1 \
                            and not BENCH_NO_P2:
                        p2_dst = ob_t[:, 0:1024] if SPLIT_JI else ob_t[:]
                        on_dve = (act_g - 1) % 25 < P2_DVE_OF \
                            if want_act else False
                        if on_dve:
                            def fmap(a_t=a_t, p2_dst=p2_dst):
                                nc.vector._custom_dve(
                                    P2_OP, out=p2_dst, in0=a_t[:])
                        else:
                            def fmap(a_t=a_t, p2_dst=p2_dst):
                                nc.gpsimd.tensor_scalar(p2_dst, a_t[:],
                                                        c_b, d_b,
                                                        OP.mult, OP.add)
                        pending.append(fmap)
                    while len(pending) > DEFER_N:
                        pending.pop(0)()
            # flush this pair's remaining pass2 ops, then batched output DMAs
            while pending:
                pending.pop(0)()
            for h2 in range(2):
                if BENCH_NO_STORE:
                    continue
                dst = d_o[2 * pair + h2, :, :].rearrange(
                    "(p j) t -> p (j t)", j=M_BLOCKS)
                dma(dst[:, :], ob[h2][:, 0:8192])


def default_key():
    """Requant constants for the reference problem's quantization params -
    used by bench.py when kernel() hasn't run in this process."""
    s_const = float(np.float32(np.float32(0.000234) / np.float32(0.0625)))
    bias_a = float(np.float64(np.float32(2.0)) - 0.5 + 2.0 ** -18)
    c_b = float(np.float32(255.0 / 256.0))
    d_b = float(np.float32(0.499))
    return (s_const, bias_a, c_b, d_b)


def kernel(x, y, alpha, a_zp, b_zp, out_zp, o_alpha):
    global LAST_RESULTS, LAST_PREP
    x = np.asarray(x)
    y = np.asarray(y)
    s_const = float(np.float32(np.float32(alpha) / np.float32(o_alpha)))
    bias_a = float(np.float64(np.float32(out_zp)) - 0.5 + 2.0 ** -18)
    c_b = float(np.float32(255.0 / 256.0))
    d_b = float(np.float32(0.499))

    # ---- host-side shard + dequant prep (exact in bf16) ----
    xf = x.reshape(B * H, S, D).astype(np.float32) - np.float32(a_zp)
    yf = y.reshape(B * H, D, T).astype(np.float32) - np.float32(b_zp)
    # lhsT layout: [head, D, S], head-pairs stacked to 128 partitions.
    # S-columns permuted to c = j*128 + p  <->  s = 8p + j so each psum
    # partition owns 8 consecutive output rows (8 KiB DMA runs).
    xt = np.ascontiguousarray(xf.transpose(0, 2, 1)).astype(ml_dtypes.bfloat16)
    xt = np.ascontiguousarray(
        xt.reshape(B * H, D, S // 8, 8).transpose(0, 1, 3, 2)).reshape(
        B * H, D, S)
    yp = yf.astype(ml_dtypes.bfloat16)
    xt = xt.reshape(N_CORES, N_PAIRS, 128, S)
    yp = yp.reshape(N_CORES, N_PAIRS, 128, T)
    # pack [xt | yp] so each pair is one input dma_start on-device
    xy = np.concatenate([xt, yp], axis=-1)

    key = (s_const, bias_a, c_b, d_b)
    if key not in _NC_CACHE:
        _NC_CACHE[key] = _build_core_program(*key)
    nc = _NC_CACHE[key]

    in_maps = [{"xy": xy[c]} for c in range(N_CORES)]
    LAST_PREP = (key, in_maps)
    res = run_bass_kernel_spmd(nc, in_maps, core_ids=list(range(N_CORES)))
    LAST_RESULTS = res

    out = np.stack([res.results[c]["o"] for c in range(N_CORES)])
    return out.reshape(B, H, S, T)


if __name__ == "__main__":
    rng = np.random.default_rng(0)
    x = rng.integers(-128, 128, size=(B, H, S, D)).astype(np.int32)
    y = rng.integers(-128, 128, size=(B, H, D, T)).astype(np.int32)
    out = kernel(x=x, y=y, alpha=np.float32(0.000234), a_zp=np.float32(3.0),
                 b_zp=np.float32(-5.0), out_zp=np.float32(2.0),
                 o_alpha=np.float32(0.0625))
    print("kernel output", out.shape, out.dtype)

